# revision 54
# baseline (speedup 1.0000x reference)
"""Cox partial-likelihood NegativeLogLikelihood loss on 8 Trainium2 cores.

reference:
    mask[i, j] = (y[j] <= y[i])                       # (N, N)
    num[j] = sum_i exp(r_i) * mask[i, j]
    den[j] = sum_i mask[i, j]
    loss = -sum_j e_j * (r_j - log(num_j / den_j)) / sum_j e_j + 0.01 * ||W||_F

Strategy: shard columns j across the 8 cores (each core owns 2048 columns).
The N x 2048 mask is materialized on-chip in [128, 2048] tiles and contracted
on the TensorEngine against lhsT = [exp_hi, exp_lo, 1, 0...] (bf16 Dekker
split, padded to 32 rows) into PSUM.

Perf structure:
  * y is re-encoded on the host as monotone bf16 codes (rank -> bf16 bit
    pattern + 0x2000), so comparisons are exact in bf16 and the DVE
    tensor_scalar(is_le) compare runs in the 4x perf mode (~0.74us per
    [128, 2048] tile vs ~1.1us for the f32 compare).
  * Row-side code copies carry a +quarter-ulp offset so code_i' > code_j
    strictly for i == j: the ScalarE Sign producer yields exactly +/-1 and
    all tie/diagonal corrections vanish.  DVE produces 93 tiles, ACT 35
    (sign-encoded with halved weights; ~2.0us/tile), balancing the engines.
  * Matmuls are column-tiled 4 ways (tile_position=(0, 32g)): four thin-M
    matmuls execute concurrently in disjoint 32-column strips of the PE
    array.  PSUM group g accumulates i-tiles 32g..32g+31 at partitions
    32g..32g+31 (rows 3..31 zero-padded so the epilogue can read PSUM
    full-width).
  * The sign-encoding corrections (+V_half into hi/lo rows, +NACT/2 per
    partition into den) are folded in by one extra matmul per jj chunk
    against an all-ones rhs -- no scalar broadcast round-trip.
  * Epilogue: PSUM -> SBUF copies chunked across scalar+vector, a selector
    matmul folds the 4 groups' rows into [num | den], 2 contiguous-dest
    scatter DMAs redistribute to [128, 16] pf layout, and each core emits
    [e_sum, w_ssq, t_sum]; the host unshard sums t over cores and applies
    -t/e + 0.01*sqrt(w_ssq).
"""
import numpy as np
import ml_dtypes
import orjson

import concourse.bass as bass
import concourse.tile as tile
import concourse.mybir as mybir
from concourse.bass_utils import run_bass_kernel_spmd

F32 = mybir.dt.float32
BF16 = mybir.dt.bfloat16

N = 16384
NCORES = 8
JSHARD = N // NCORES            # 2048 columns per core
NT = N // 128                   # 128 i-tiles of 128 rows
NG = 4                          # PE column-strip groups
NR = NT // NG                   # 32 i-tiles (rounds) per group
NJJ = JSHARD // 512             # 4 matmul column chunks per core
NACT = 36                       # ACT-produced tiles (sign-encoded)
DEN_ROW = float(NACT) / 2.0     # per-partition den correction row


def tile_of(g, r):
    return 32 * g + r


def is_act(g, r):
    # ACT owns group 3 rounds 0..26 plus mid-stream extras in groups 1-2.
    # No ACT tiles in rounds >= 27: the serial ~1.9us/tile ACT stream would
    # otherwise gate the PE through the entire endgame.
    return ((g == 3 and r <= 26)
            or (g == 2 and r in (2, 5, 10, 15, 20, 25))
            or (g == 1 and r in (7, 14, 21)))


ACT_TILES = [tile_of(g, r) for g in range(NG) for r in range(NR) if is_act(g, r)]

# ---------------------------------------------------------------------------
# Workaround for the installed walrus accepting at most ONE sync-wait command
# per TPB instruction: split multi-wait instructions into preceding
# single-wait EventSemaphore instructions on the same engine.
# ---------------------------------------------------------------------------


def _fix_bir_multiwait(bir_json: bytes) -> bytes:
    d = orjson.loads(bir_json)
    counter = 0
    for fn in d.get("functions", []):
        stack = list(fn.get("blocks", []))
        while stack:
            block = stack.pop()
            stack.extend(block.get("blocks", []))
            new_insts = []
            for inst in block.get("instructions", []):
                sync = inst.get("sync_info") or {}
                waits = sync.get("on_wait") or []
                if len(waits) > 1:
                    for w in waits[:-1]:
                        counter += 1
                        new_insts.append({
                            "debug": inst.get("debug", 0),
                            "engine": inst.get("engine"),
                            "ins": [],
                            "name": f"esw_fix_{counter}",
                            "opcode": "EventSemaphore",
                            "outs": [],
                            "sync_info": {"on_update": [], "on_wait": [w]},
                        })
                    sync["on_wait"] = [waits[-1]]
                new_insts.append(inst)
            block["instructions"] = new_insts
    return orjson.dumps(d)


_patched = False


def _install_bir_fix():
    global _patched
    if _patched:
        return
    _patched = True
    import concourse.bass_utils as bu
    import concourse.bass2jax as b2j

    orig = bu.compile_bir_kernel

    def patched(bir_json, tmpdir, neff_name="file.neff"):
        if isinstance(bir_json, str):
            bir_json = bir_json.encode()
        return orig(_fix_bir_multiwait(bir_json), tmpdir, neff_name)

    bu.compile_bir_kernel = patched
    b2j.compile_bir_kernel = patched


# ---------------------------------------------------------------------------
# Kernel build
# ---------------------------------------------------------------------------

def build_kernel() -> bass.Bass:
    nc = bass.Bass()
    Sign = mybir.ActivationFunctionType.Sign

    # j-side codes, host-broadcast to all 128 partitions, bf16
    yb_d = nc.dram_tensor("yb", [128, JSHARD], BF16, kind="ExternalInput")
    # crit: the columns every producer needs early
    CRIT_W = NT + NT + NT + NT
    crit = nc.dram_tensor("crit", [128, CRIT_W], F32, kind="ExternalInput")
    # rest: [r_pf | e_pf | e_f | w | sel]
    REST_W = 16 + 16 + NT + 1024 + 2
    rest = nc.dram_tensor("rest", [128, REST_W], F32, kind="ExternalInput")
    out = nc.dram_tensor("out", [1, 3], F32, kind="ExternalOutput")

    with tile.TileContext(nc) as tc:
        with (
            tc.tile_pool(name="const", bufs=1) as const,
            tc.tile_pool(name="masks", bufs=22) as masks,
            tc.tile_pool(name="psacc", bufs=1, space="PSUM") as psacc,
            tc.tile_pool(name="psaux", bufs=1, space="PSUM") as psaux,
        ):
            # ---- DVE-local init first (no input deps; overlaps the DMAs)
            ones_col = const.tile([128, 1], F32)
            nc.vector.memset(ones_col, 1.0)
            zeros_32 = const.tile([128, 32], BF16)
            nc.vector.memset(zeros_32, 0.0)
            ones_b = const.tile([128, 512], BF16)
            nc.vector.memset(ones_b, 1.0)
            lhsT = const.tile([128, 3, NT], BF16)

            # ---- critical-path loads (hw-dge queues: sync/scalar; the
            # gpsimd software-dge queue is ~10x slower per transfer).
            # crit goes first: exp/Dekker depend on it alone.
            crit_sb = const.tile([128, CRIT_W], F32)
            nc.sync.dma_start(out=crit_sb, in_=crit[:, :])
            yb = const.tile([128, JSHARD], BF16)
            nc.sync.dma_start(out=yb, in_=yb_d[:, :])
            o = 0
            ycol_sb = crit_sb[:, o:o + NT]; o += NT    # code_i + delta (f32)
            rcol_sb = crit_sb[:, o:o + NT]; o += NT    # risk_pred col-major
            scale_b = crit_sb[:, o:o + NT]; o += NT    # 0.5 on ACT cols else 1
            indh_b = crit_sb[:, o:o + NT]; o += NT     # 0.5 on ACT cols else 0
            rest_sb = const.tile([128, REST_W], F32)
            nc.scalar.dma_start(out=rest_sb, in_=rest[:, :])
            o = 0
            rpf_sb = rest_sb[:, o:o + 16]; o += 16
            epf_f = rest_sb[:, o:o + 16]; o += 16
            e_f = rest_sb[:, o:o + NT]; o += NT
            w_sb = rest_sb[:, o:o + 1024]; o += 1024
            sel_sb = rest_sb[:, o:o + 2]; o += 2       # group-fold selectors

            exp_sb = const.tile([128, NT], F32)
            nc.scalar.activation(exp_sb, rcol_sb, mybir.ActivationFunctionType.Exp)

            # ---- main loop: mask tiles + column-tiled matmul accumulation
            # acc group g lives at partitions [32g, 32g+32), banks by jj chunk
            acc = psacc.tile([128, NJJ * 512], F32)

            # start-of-chain init matmuls: zero weights, start=True.  They
            # zero ALL 32 partitions of each group's PSUM block and set
            # has_written, so the thin [3, 512] main matmuls can accumulate
            # with start=False and the epilogue can read PSUM full-width.
            # (Also serves as the PE HAM warm-up.)
            for g in range(NG):
                for jj in range(NJJ):
                    nc.tensor.matmul(
                        acc[32 * g:32 * g + 32, 512 * jj:512 * (jj + 1)],
                        zeros_32, ones_b,
                        start=True, stop=False,
                        tile_position=(0, 32 * g), skip_group_check=True,
                    )

            lhsT_va = const.tile([128, 32], BF16)
            vh = const.tile([128, NT], F32)
            vred = const.tile([128, 1], F32)
            vhi32 = const.tile([128, 1], F32)
            vlo = const.tile([128, 1], F32)
            hi32 = const.tile([128, NT], F32)
            lo32 = const.tile([128, NT], F32)
            vec3 = const.tile([128, 3], F32)

            for r in range(NR):
                mt = {}
                for g in range(NG):
                    t = tile_of(g, r)
                    m = masks.tile([128, JSHARD], BF16)
                    mt[g] = m
                    if is_act(g, r):
                        nc.scalar.activation(
                            m, yb, Sign, bias=ycol_sb[:, t:t + 1], scale=-1.0,
                        )
                    else:
                        nc.vector.tensor_scalar(
                            out=m, in0=yb,
                            scalar1=ycol_sb[:, t:t + 1], scalar2=None,
                            op0=mybir.AluOpType.is_le,
                        )
                if r == 0:
                    # lhsT rows 0..2 = scale * [exp_hi | exp_lo | ones], bf16
                    # (emitted after the first masks so they head the DVE
                    # queue, but before any matmul reads lhsT)
                    nc.vector.tensor_copy(lhsT[:, 0, :], exp_sb)   # bf16(exp)
                    nc.vector.tensor_copy(hi32, lhsT[:, 0, :])     # back to f32
                    nc.vector.tensor_sub(lo32, exp_sb, hi32)       # f32 residual
                    nc.vector.tensor_mul(lhsT[:, 0, :], hi32, scale_b)
                    nc.vector.tensor_mul(lhsT[:, 1, :], lo32, scale_b)
                    nc.vector.tensor_copy(lhsT[:, 2, :], scale_b)
                for g in range(NG):
                    t = tile_of(g, r)
                    for jj in range(NJJ):
                        nc.tensor.matmul(
                            acc[32 * g:32 * g + 3, 512 * jj:512 * (jj + 1)],
                            lhsT[:, :, t],
                            mt[g][:, 512 * jj:512 * (jj + 1)],
                            start=False,
                            stop=(r == NR - 1),
                            tile_position=(0, 32 * g),
                            skip_group_check=True,
                        )
                # deferred off-critical DVE prologue work (keeps the first
                # mask tiles at the head of the DVE queue)
                if r == 2:
                    nc.vector.memset(lhsT_va, 0.0)
                    nc.vector.tensor_mul(vh, exp_sb, indh_b)
                    nc.vector.tensor_reduce(
                        out=vred, in_=vh, axis=mybir.AxisListType.X,
                        op=mybir.AluOpType.add)
                elif r == 3:
                    nc.vector.tensor_copy(lhsT_va[:, 0:1], vred)  # vred_hi
                    nc.vector.tensor_copy(vhi32, lhsT_va[:, 0:1])
                    nc.vector.tensor_sub(vlo, vred, vhi32)
                    nc.vector.tensor_copy(lhsT_va[:, 1:2], vlo)   # vred_lo
                    nc.vector.memset(lhsT_va[:, 2:3], DEN_ROW)
                elif r == 8:
                    nc.vector.tensor_reduce(
                        out=vec3[:, 0:1], in_=e_f, axis=mybir.AxisListType.X,
                        op=mybir.AluOpType.add)
                elif r == 6:
                    # correction matmuls, mid-stream: add [V_half_hi,
                    # V_half_lo, NACT/2] (summed over partitions by the PE)
                    # into the group-0 rows for every column
                    for jj in range(NJJ):
                        nc.tensor.matmul(
                            acc[0:32, 512 * jj:512 * (jj + 1)],
                            lhsT_va, ones_b,
                            start=False, stop=False,
                            tile_position=(0, 0), skip_group_check=True,
                        )

            # ---- late ACT-side work that overlaps the epilogue
            w2d = const.tile([128, 1024], F32)
            nc.scalar.activation(
                w2d, w_sb, mybir.ActivationFunctionType.Square,
                accum_out=vec3[:, 1:2],
            )

            # ---- epilogue: PSUM -> SBUF staging (chunked, both engines),
            # selector matmul folds 4 groups' (hi+lo) and den rows into
            # [num | den], 2 contiguous-dest scatter DMAs into pf layout.
            # pf mapping: x_pf[p, c] = x_shard[16*p + c]
            F32R = mybir.dt.float32r
            sel_r = const.tile([128, 2], F32R)
            nc.vector.tensor_copy(sel_r, sel_sb)
            nd_all = const.tile([128, NJJ * 512], F32R)
            fold = psaux.tile([2, NJJ * 512], F32, name="fold")
            for jj in range(NJJ):
                if jj % 2 == 0:
                    nc.scalar.copy(nd_all[:, 512 * jj:512 * (jj + 1)],
                                   acc[:, 512 * jj:512 * (jj + 1)])
                else:
                    nc.vector.tensor_copy(nd_all[:, 512 * jj:512 * (jj + 1)],
                                          acc[:, 512 * jj:512 * (jj + 1)])
                # f32r: single-pass reduced-precision fp32 matmul (fp22
                # mantissa -- plenty for the 2e-2 budget, 2x faster)
                nc.tensor.matmul(
                    fold[:, 512 * jj:512 * (jj + 1)],
                    sel_r, nd_all[:, 512 * jj:512 * (jj + 1)],
                    start=True, stop=True, skip_group_check=True,
                )
            nd2 = const.tile([2, NJJ * 512], F32)
            nc.scalar.copy(nd2[:, 0:1024], fold[:, 0:1024])
            nc.vector.tensor_copy(nd2[:, 1024:2048], fold[:, 1024:2048])
            num_pf = const.tile([128, 16], F32)
            den_pf = const.tile([128, 16], F32)
            nc.sync.dma_start(out=num_pf, in_=nd2[0:1, :])
            nc.scalar.dma_start(out=den_pf, in_=nd2[1:2, :])

            # ---- wide final math on [128, 16] (den first: its scatter is
            # on the scalar queue, so the Ln needs no cross-engine hop)
            lnd = const.tile([128, 16], F32)
            nc.scalar.activation(lnd, den_pf, mybir.ActivationFunctionType.Ln)
            lnn = const.tile([128, 16], F32)
            nc.scalar.activation(lnn, num_pf, mybir.ActivationFunctionType.Ln)
            s1 = const.tile([128, 16], F32)
            nc.vector.tensor_sub(s1, rpf_sb, lnn)
            s2 = const.tile([128, 16], F32)
            nc.vector.scalar_tensor_tensor(
                out=s2, in0=s1, scalar=1.0, in1=lnd,
                op0=mybir.AluOpType.mult, op1=mybir.AluOpType.add)
            s3 = const.tile([128, 16], F32)
            nc.vector.scalar_tensor_tensor(
                out=s3, in0=s2, scalar=1.0, in1=epf_f,
                op0=mybir.AluOpType.mult, op1=mybir.AluOpType.mult,
                accum_out=vec3[:, 2:3])

            # ---- cross-partition fold: [e_sum, w_ssq, t_sum] into one row
            # (reuses a slice of the fold tile; WAR deps order it after nd2)
            sums = fold[0:1, 0:3]
            nc.tensor.matmul(sums, ones_col, vec3[:, :], start=True,
                             stop=True, skip_group_check=True)
            res3 = const.tile([1, 3], F32)
            nc.scalar.copy(res3, sums)
            nc.scalar.dma_start(out=out[:, :], in_=res3)

    return nc


_nc_cache = None


def _get_nc():
    global _nc_cache
    if _nc_cache is None:
        _install_bir_fix()
        _nc_cache = build_kernel()
    return _nc_cache


def make_in_maps(risk_pred, y, e, W):
    """Host-side sharding: slice/reshape/encode the full inputs per core."""
    yflat = y.reshape(-1)
    # monotone distinct bf16 codes: rank -> bf16 bit pattern (+0x2000 keeps
    # every code and its successor a normal number in [2^-63, 2^64], so
    # all pairwise differences are far from f32 under/overflow)
    order = np.argsort(yflat, kind="stable")
    ranks = np.empty(N, np.uint16)
    ranks[order] = np.arange(N, dtype=np.uint16)
    codes_u16 = (ranks + np.uint16(0x2000)).astype(np.uint16)
    codes_bf16 = codes_u16.view(ml_dtypes.bfloat16)
    codes_f32 = codes_bf16.astype(np.float32)
    nxt_f32 = (codes_u16 + np.uint16(1)).view(ml_dtypes.bfloat16).astype(np.float32)
    # row-side codes get +quarter-gap so the diagonal compare is strict (+1)
    ycol_delta = codes_f32 + 0.25 * (nxt_f32 - codes_f32)

    ycol = ycol_delta.reshape(NT, 128).T                     # [p, t]
    rcol = risk_pred.reshape(NT, 128).T.astype(np.float32)
    ef = e.astype(np.float32).reshape(NT, 128).T
    w_flat = W.reshape(128, 1024).astype(np.float32)
    act_mask = np.zeros(NT, np.float32)
    act_mask[ACT_TILES] = 1.0
    scale_b = np.tile(1.0 - 0.5 * act_mask, (128, 1)).astype(np.float32)
    indh_b = np.tile(0.5 * act_mask, (128, 1)).astype(np.float32)
    # group-fold selector: col 0 sums the hi+lo rows (p%32 in {0,1}) of the
    # 4 PSUM groups, col 1 sums the den rows (p%32 == 2)
    p = np.arange(128)
    sel = np.stack([((p % 32) <= 1), ((p % 32) == 2)], axis=1).astype(np.float32)

    crit = np.ascontiguousarray(np.concatenate(
        [ycol, rcol, scale_b, indh_b], axis=1), dtype=np.float32)

    in_maps = []
    for c in range(NCORES):
        j0 = c * JSHARD
        rsh = risk_pred.reshape(-1)[j0:j0 + JSHARD]
        esh = e.astype(np.float32).reshape(-1)[j0:j0 + JSHARD]
        r_pf = rsh.reshape(128, 16).astype(np.float32)
        e_pf = esh.reshape(128, 16)
        rest = np.ascontiguousarray(np.concatenate(
            [r_pf, e_pf, ef, w_flat, sel], axis=1), dtype=np.float32)
        yb = np.ascontiguousarray(
            np.broadcast_to(codes_bf16[j0:j0 + JSHARD], (128, JSHARD)))
        in_maps.append(dict(yb=yb, crit=crit, rest=rest))
    return in_maps


def kernel(risk_pred, y, e, W, **run_kwargs):
    nc = _get_nc()
    in_maps = make_in_maps(
        np.asarray(risk_pred, np.float32),
        np.asarray(y, np.float32),
        np.asarray(e, np.int32),
        np.asarray(W, np.float32),
    )
    result = run_bass_kernel_spmd(nc, in_maps, core_ids=list(range(NCORES)),
                                  **run_kwargs)
    # gather/unshard: t_sum adds across cores; e_sum and w_ssq are computed
    # from replicated inputs (identical on every core)
    t_total = np.float32(0.0)
    for r in result.results:
        t_total = np.float32(t_total + r["out"][0, 2])
    e_sum = np.float32(result.results[0]["out"][0, 0])
    w_ssq = np.float32(result.results[0]["out"][0, 1])
    total = np.float32(-t_total / e_sum + np.float32(0.01) * np.sqrt(w_ssq))
    kernel.last_result = result
    return np.asarray(total, np.float32)


# revision 58
# speedup vs baseline: 1.0120x; 1.0120x over previous
"""Cox partial-likelihood NegativeLogLikelihood loss on 8 Trainium2 cores.

reference:
    mask[i, j] = (y[j] <= y[i])                       # (N, N)
    num[j] = sum_i exp(r_i) * mask[i, j]
    den[j] = sum_i mask[i, j]
    loss = -sum_j e_j * (r_j - log(num_j / den_j)) / sum_j e_j + 0.01 * ||W||_F

Strategy: shard columns j across the 8 cores (each core owns 2048 columns).
The N x 2048 mask is materialized on-chip in [128, 2048] tiles and contracted
on the TensorEngine against lhsT = [exp_hi, exp_lo, 1, 0...] (bf16 Dekker
split, padded to 32 rows) into PSUM.

Perf structure:
  * y is re-encoded on the host as monotone bf16 codes (rank -> bf16 bit
    pattern + 0x2000), so comparisons are exact in bf16 and the DVE
    tensor_scalar(is_le) compare runs in the 4x perf mode (~0.74us per
    [128, 2048] tile vs ~1.1us for the f32 compare).
  * Row-side code copies carry a +quarter-ulp offset so code_i' > code_j
    strictly for i == j: the ScalarE Sign producer yields exactly +/-1 and
    all tie/diagonal corrections vanish.  DVE produces 93 tiles, ACT 35
    (sign-encoded with halved weights; ~2.0us/tile), balancing the engines.
  * Matmuls are column-tiled 4 ways (tile_position=(0, 32g)): four thin-M
    matmuls execute concurrently in disjoint 32-column strips of the PE
    array.  PSUM group g accumulates i-tiles 32g..32g+31 at partitions
    32g..32g+31 (rows 3..31 zero-padded so the epilogue can read PSUM
    full-width).
  * The sign-encoding corrections (+V_half into hi/lo rows, +NACT/2 per
    partition into den) are folded in by one extra matmul per jj chunk
    against an all-ones rhs -- no scalar broadcast round-trip.
  * Epilogue: PSUM -> SBUF copies chunked across scalar+vector, a selector
    matmul folds the 4 groups' rows into [num | den], 2 contiguous-dest
    scatter DMAs redistribute to [128, 16] pf layout, and each core emits
    [e_sum, w_ssq, t_sum]; the host unshard sums t over cores and applies
    -t/e + 0.01*sqrt(w_ssq).
"""
import numpy as np
import ml_dtypes
import orjson

import concourse.bass as bass
import concourse.tile as tile
import concourse.mybir as mybir
from concourse.bass_utils import run_bass_kernel_spmd

F32 = mybir.dt.float32
BF16 = mybir.dt.bfloat16

N = 16384
NCORES = 8
JSHARD = N // NCORES            # 2048 columns per core
NT = N // 128                   # 128 i-tiles of 128 rows
NG = 4                          # PE column-strip groups
NR = NT // NG                   # 32 i-tiles (rounds) per group
NJJ = JSHARD // 512             # 4 matmul column chunks per core
NACT = 34                       # ACT-produced tiles (sign-encoded)
DEN_ROW = float(NACT) / 2.0     # per-partition den correction row


def tile_of(g, r):
    return 32 * g + r


def is_act(g, r):
    # ACT owns group 3 rounds 0..26 plus mid-stream extras in groups 1-2.
    # No ACT tiles in rounds >= 27: the serial ~1.9us/tile ACT stream would
    # otherwise gate the PE through the entire endgame.
    return ((g == 3 and r <= 26)
            or (g == 2 and r in (2, 10, 18, 25))
            or (g == 1 and r in (7, 14, 21)))


ACT_TILES = [tile_of(g, r) for g in range(NG) for r in range(NR) if is_act(g, r)]

# ---------------------------------------------------------------------------
# Workaround for the installed walrus accepting at most ONE sync-wait command
# per TPB instruction: split multi-wait instructions into preceding
# single-wait EventSemaphore instructions on the same engine.
# ---------------------------------------------------------------------------


def _fix_bir_multiwait(bir_json: bytes) -> bytes:
    d = orjson.loads(bir_json)
    counter = 0
    for fn in d.get("functions", []):
        stack = list(fn.get("blocks", []))
        while stack:
            block = stack.pop()
            stack.extend(block.get("blocks", []))
            new_insts = []
            for inst in block.get("instructions", []):
                sync = inst.get("sync_info") or {}
                waits = sync.get("on_wait") or []
                if len(waits) > 1:
                    for w in waits[:-1]:
                        counter += 1
                        new_insts.append({
                            "debug": inst.get("debug", 0),
                            "engine": inst.get("engine"),
                            "ins": [],
                            "name": f"esw_fix_{counter}",
                            "opcode": "EventSemaphore",
                            "outs": [],
                            "sync_info": {"on_update": [], "on_wait": [w]},
                        })
                    sync["on_wait"] = [waits[-1]]
                new_insts.append(inst)
            block["instructions"] = new_insts
    return orjson.dumps(d)


_patched = False


def _install_bir_fix():
    global _patched
    if _patched:
        return
    _patched = True
    import concourse.bass_utils as bu
    import concourse.bass2jax as b2j

    orig = bu.compile_bir_kernel

    def patched(bir_json, tmpdir, neff_name="file.neff"):
        if isinstance(bir_json, str):
            bir_json = bir_json.encode()
        return orig(_fix_bir_multiwait(bir_json), tmpdir, neff_name)

    bu.compile_bir_kernel = patched
    b2j.compile_bir_kernel = patched


# ---------------------------------------------------------------------------
# Kernel build
# ---------------------------------------------------------------------------

def build_kernel() -> bass.Bass:
    nc = bass.Bass()
    Sign = mybir.ActivationFunctionType.Sign

    # j-side codes, host-broadcast to all 128 partitions, bf16
    yb_d = nc.dram_tensor("yb", [128, JSHARD], BF16, kind="ExternalInput")
    # crit: the columns every producer needs early
    CRIT_W = NT + NT + NT + NT
    crit = nc.dram_tensor("crit", [128, CRIT_W], F32, kind="ExternalInput")
    # rest: [r_pf | e_pf | e_f | w | sel]
    REST_W = 16 + 16 + NT + 1024 + 2
    rest = nc.dram_tensor("rest", [128, REST_W], F32, kind="ExternalInput")
    out = nc.dram_tensor("out", [1, 3], F32, kind="ExternalOutput")

    with tile.TileContext(nc) as tc:
        with (
            tc.tile_pool(name="const", bufs=1) as const,
            tc.tile_pool(name="masks", bufs=26) as masks,
            tc.tile_pool(name="psacc", bufs=1, space="PSUM") as psacc,
            tc.tile_pool(name="psaux", bufs=1, space="PSUM") as psaux,
        ):
            # ---- DVE-local init first (no input deps; overlaps the DMAs)
            ones_col = const.tile([128, 1], F32)
            nc.vector.memset(ones_col, 1.0)
            zeros_32 = const.tile([128, 32], BF16)
            nc.vector.memset(zeros_32, 0.0)
            ones_b = const.tile([128, 512], BF16)
            nc.vector.memset(ones_b, 1.0)
            lhsT = const.tile([128, 3, NT], BF16)

            # ---- critical-path loads on SEPARATE hw-dge queues (the gpsimd
            # software-dge queue is ~10x slower per transfer; two loads on
            # one queue serialize their transfers)
            yb = const.tile([128, JSHARD], BF16)
            nc.sync.dma_start(out=yb, in_=yb_d[:, :])
            crit_sb = const.tile([128, CRIT_W], F32)
            nc.scalar.dma_start(out=crit_sb, in_=crit[:, :])
            o = 0
            ycol_sb = crit_sb[:, o:o + NT]; o += NT    # code_i + delta (f32)
            rcol_sb = crit_sb[:, o:o + NT]; o += NT    # risk_pred col-major
            scale_b = crit_sb[:, o:o + NT]; o += NT    # 0.5 on ACT cols else 1
            indh_b = crit_sb[:, o:o + NT]; o += NT     # 0.5 on ACT cols else 0
            rest_sb = const.tile([128, REST_W], F32)
            nc.scalar.dma_start(out=rest_sb, in_=rest[:, :])
            o = 0
            rpf_sb = rest_sb[:, o:o + 16]; o += 16
            epf_f = rest_sb[:, o:o + 16]; o += 16
            e_f = rest_sb[:, o:o + NT]; o += NT
            w_sb = rest_sb[:, o:o + 1024]; o += 1024
            sel_sb = rest_sb[:, o:o + 2]; o += 2       # group-fold selectors

            exp_sb = const.tile([128, NT], F32)
            nc.scalar.activation(exp_sb, rcol_sb, mybir.ActivationFunctionType.Exp)

            # ---- main loop: mask tiles + column-tiled matmul accumulation
            # acc group g lives at partitions [32g, 32g+32), banks by jj chunk
            acc = psacc.tile([128, NJJ * 512], F32)

            # start-of-chain init matmuls: zero weights, start=True.  They
            # zero ALL 32 partitions of each group's PSUM block and set
            # has_written, so the thin [3, 512] main matmuls can accumulate
            # with start=False and the epilogue can read PSUM full-width.
            # (Also serves as the PE HAM warm-up.)
            for g in range(NG):
                for jj in range(NJJ):
                    nc.tensor.matmul(
                        acc[32 * g:32 * g + 32, 512 * jj:512 * (jj + 1)],
                        zeros_32, ones_b,
                        start=True, stop=False,
                        tile_position=(0, 32 * g), skip_group_check=True,
                    )

            lhsT_va = const.tile([128, 32], BF16)
            vh = const.tile([128, NT], F32)
            vred = const.tile([128, 1], F32)
            vhi32 = const.tile([128, 1], F32)
            vlo = const.tile([128, 1], F32)
            hi32 = const.tile([128, NT], F32)
            lo32 = const.tile([128, NT], F32)
            vec3 = const.tile([128, 3], F32)

            for r in range(NR):
                mt = {}
                for g in range(NG):
                    t = tile_of(g, r)
                    m = masks.tile([128, JSHARD], BF16)
                    mt[g] = m
                    if is_act(g, r):
                        nc.scalar.activation(
                            m, yb, Sign, bias=ycol_sb[:, t:t + 1], scale=-1.0,
                        )
                    else:
                        nc.vector.tensor_scalar(
                            out=m, in0=yb,
                            scalar1=ycol_sb[:, t:t + 1], scalar2=None,
                            op0=mybir.AluOpType.is_le,
                        )
                if r == 0:
                    # lhsT rows 0..2 = scale * [exp_hi | exp_lo | ones], bf16
                    # (emitted after the first masks so they head the DVE
                    # queue, but before any matmul reads lhsT)
                    nc.vector.tensor_copy(lhsT[:, 0, :], exp_sb)   # bf16(exp)
                    nc.vector.tensor_copy(hi32, lhsT[:, 0, :])     # back to f32
                    nc.vector.tensor_sub(lo32, exp_sb, hi32)       # f32 residual
                    nc.vector.tensor_mul(lhsT[:, 0, :], hi32, scale_b)
                    nc.vector.tensor_mul(lhsT[:, 1, :], lo32, scale_b)
                    nc.vector.tensor_copy(lhsT[:, 2, :], scale_b)
                for g in range(NG):
                    t = tile_of(g, r)
                    for jj in range(NJJ):
                        nc.tensor.matmul(
                            acc[32 * g:32 * g + 3, 512 * jj:512 * (jj + 1)],
                            lhsT[:, :, t],
                            mt[g][:, 512 * jj:512 * (jj + 1)],
                            start=False,
                            stop=(r == NR - 1),
                            tile_position=(0, 32 * g),
                            skip_group_check=True,
                        )
                # deferred off-critical DVE prologue work (keeps the first
                # mask tiles at the head of the DVE queue)
                if r == 2:
                    nc.vector.memset(lhsT_va, 0.0)
                    nc.vector.tensor_mul(vh, exp_sb, indh_b)
                    nc.vector.tensor_reduce(
                        out=vred, in_=vh, axis=mybir.AxisListType.X,
                        op=mybir.AluOpType.add)
                elif r == 3:
                    nc.vector.tensor_copy(lhsT_va[:, 0:1], vred)  # vred_hi
                    nc.vector.tensor_copy(vhi32, lhsT_va[:, 0:1])
                    nc.vector.tensor_sub(vlo, vred, vhi32)
                    nc.vector.tensor_copy(lhsT_va[:, 1:2], vlo)   # vred_lo
                    nc.vector.memset(lhsT_va[:, 2:3], DEN_ROW)
                elif r == 8:
                    nc.vector.tensor_reduce(
                        out=vec3[:, 0:1], in_=e_f, axis=mybir.AxisListType.X,
                        op=mybir.AluOpType.add)
                elif r == 6:
                    # correction matmuls, mid-stream: add [V_half_hi,
                    # V_half_lo, NACT/2] (summed over partitions by the PE)
                    # into the group-0 rows for every column
                    for jj in range(NJJ):
                        nc.tensor.matmul(
                            acc[0:32, 512 * jj:512 * (jj + 1)],
                            lhsT_va, ones_b,
                            start=False, stop=False,
                            tile_position=(0, 0), skip_group_check=True,
                        )

            # ---- late ACT-side work that overlaps the epilogue
            w2d = const.tile([128, 1024], F32)
            nc.scalar.activation(
                w2d, w_sb, mybir.ActivationFunctionType.Square,
                accum_out=vec3[:, 1:2],
            )

            # ---- epilogue: PSUM -> SBUF staging (chunked, both engines),
            # selector matmul folds 4 groups' (hi+lo) and den rows into
            # [num | den], 2 contiguous-dest scatter DMAs into pf layout.
            # pf mapping: x_pf[p, c] = x_shard[16*p + c]
            F32R = mybir.dt.float32r
            sel_r = const.tile([128, 2], F32R)
            nc.vector.tensor_copy(sel_r, sel_sb)
            nd_all = const.tile([128, NJJ * 512], F32R)
            fold = psaux.tile([2, NJJ * 512], F32, name="fold")
            for jj in range(NJJ):
                if jj % 2 == 0:
                    nc.scalar.copy(nd_all[:, 512 * jj:512 * (jj + 1)],
                                   acc[:, 512 * jj:512 * (jj + 1)])
                else:
                    nc.vector.tensor_copy(nd_all[:, 512 * jj:512 * (jj + 1)],
                                          acc[:, 512 * jj:512 * (jj + 1)])
                # f32r: single-pass reduced-precision fp32 matmul (fp22
                # mantissa -- plenty for the 2e-2 budget, 2x faster)
                nc.tensor.matmul(
                    fold[:, 512 * jj:512 * (jj + 1)],
                    sel_r, nd_all[:, 512 * jj:512 * (jj + 1)],
                    start=True, stop=True, skip_group_check=True,
                )
            # Ln straight off the PSUM fold rows (both rows per chunk: the
            # ACT cost is free-size-driven), then scatter the LN'd rows
            lnnd = const.tile([2, NJJ * 512], F32)
            nc.scalar.activation(lnnd[:, 0:1024], fold[:, 0:1024],
                                 mybir.ActivationFunctionType.Ln)
            nc.scalar.activation(lnnd[:, 1024:2048], fold[:, 1024:2048],
                                 mybir.ActivationFunctionType.Ln)
            lnn = const.tile([128, 16], F32)
            lnd = const.tile([128, 16], F32)
            nc.sync.dma_start(out=lnn, in_=lnnd[0:1, :])
            nc.scalar.dma_start(out=lnd, in_=lnnd[1:2, :])

            # ---- wide final math on [128, 16]
            s1 = const.tile([128, 16], F32)
            nc.vector.tensor_sub(s1, rpf_sb, lnn)
            s2 = const.tile([128, 16], F32)
            nc.vector.scalar_tensor_tensor(
                out=s2, in0=s1, scalar=1.0, in1=lnd,
                op0=mybir.AluOpType.mult, op1=mybir.AluOpType.add)
            s3 = const.tile([128, 16], F32)
            nc.vector.scalar_tensor_tensor(
                out=s3, in0=s2, scalar=1.0, in1=epf_f,
                op0=mybir.AluOpType.mult, op1=mybir.AluOpType.mult,
                accum_out=vec3[:, 2:3])

            # ---- cross-partition fold: [e_sum, w_ssq, t_sum] into one row
            # (reuses a slice of the fold tile; WAR deps order it after nd2)
            sums = fold[0:1, 0:3]
            nc.tensor.matmul(sums, ones_col, vec3[:, :], start=True,
                             stop=True, skip_group_check=True)
            res3 = const.tile([1, 3], F32)
            nc.scalar.copy(res3, sums)
            nc.scalar.dma_start(out=out[:, :], in_=res3)

    return nc


_nc_cache = None


def _get_nc():
    global _nc_cache
    if _nc_cache is None:
        _install_bir_fix()
        _nc_cache = build_kernel()
    return _nc_cache


def make_in_maps(risk_pred, y, e, W):
    """Host-side sharding: slice/reshape/encode the full inputs per core."""
    yflat = y.reshape(-1)
    # monotone distinct bf16 codes: rank -> bf16 bit pattern (+0x2000 keeps
    # every code and its successor a normal number in [2^-63, 2^64], so
    # all pairwise differences are far from f32 under/overflow)
    order = np.argsort(yflat, kind="stable")
    ranks = np.empty(N, np.uint16)
    ranks[order] = np.arange(N, dtype=np.uint16)
    codes_u16 = (ranks + np.uint16(0x2000)).astype(np.uint16)
    codes_bf16 = codes_u16.view(ml_dtypes.bfloat16)
    codes_f32 = codes_bf16.astype(np.float32)
    nxt_f32 = (codes_u16 + np.uint16(1)).view(ml_dtypes.bfloat16).astype(np.float32)
    # row-side codes get +quarter-gap so the diagonal compare is strict (+1)
    ycol_delta = codes_f32 + 0.25 * (nxt_f32 - codes_f32)

    ycol = ycol_delta.reshape(NT, 128).T                     # [p, t]
    rcol = risk_pred.reshape(NT, 128).T.astype(np.float32)
    ef = e.astype(np.float32).reshape(NT, 128).T
    w_flat = W.reshape(128, 1024).astype(np.float32)
    act_mask = np.zeros(NT, np.float32)
    act_mask[ACT_TILES] = 1.0
    scale_b = np.tile(1.0 - 0.5 * act_mask, (128, 1)).astype(np.float32)
    indh_b = np.tile(0.5 * act_mask, (128, 1)).astype(np.float32)
    # group-fold selector: col 0 sums the hi+lo rows (p%32 in {0,1}) of the
    # 4 PSUM groups, col 1 sums the den rows (p%32 == 2)
    p = np.arange(128)
    sel = np.stack([((p % 32) <= 1), ((p % 32) == 2)], axis=1).astype(np.float32)

    crit = np.ascontiguousarray(np.concatenate(
        [ycol, rcol, scale_b, indh_b], axis=1), dtype=np.float32)

    in_maps = []
    for c in range(NCORES):
        j0 = c * JSHARD
        rsh = risk_pred.reshape(-1)[j0:j0 + JSHARD]
        esh = e.astype(np.float32).reshape(-1)[j0:j0 + JSHARD]
        r_pf = rsh.reshape(128, 16).astype(np.float32)
        e_pf = esh.reshape(128, 16)
        rest = np.ascontiguousarray(np.concatenate(
            [r_pf, e_pf, ef, w_flat, sel], axis=1), dtype=np.float32)
        yb = np.ascontiguousarray(
            np.broadcast_to(codes_bf16[j0:j0 + JSHARD], (128, JSHARD)))
        in_maps.append(dict(yb=yb, crit=crit, rest=rest))
    return in_maps


def kernel(risk_pred, y, e, W, **run_kwargs):
    nc = _get_nc()
    in_maps = make_in_maps(
        np.asarray(risk_pred, np.float32),
        np.asarray(y, np.float32),
        np.asarray(e, np.int32),
        np.asarray(W, np.float32),
    )
    result = run_bass_kernel_spmd(nc, in_maps, core_ids=list(range(NCORES)),
                                  **run_kwargs)
    # gather/unshard: t_sum adds across cores; e_sum and w_ssq are computed
    # from replicated inputs (identical on every core)
    t_total = np.float32(0.0)
    for r in result.results:
        t_total = np.float32(t_total + r["out"][0, 2])
    e_sum = np.float32(result.results[0]["out"][0, 0])
    w_ssq = np.float32(result.results[0]["out"][0, 1])
    total = np.float32(-t_total / e_sum + np.float32(0.01) * np.sqrt(w_ssq))
    kernel.last_result = result
    return np.asarray(total, np.float32)


# revision 64
# speedup vs baseline: 1.0789x; 1.0660x over previous
"""Cox partial-likelihood NegativeLogLikelihood loss on 8 Trainium2 cores.

reference:
    mask[i, j] = (y[j] <= y[i])                       # (N, N)
    num[j] = sum_i exp(r_i) * mask[i, j]
    den[j] = sum_i mask[i, j]
    loss = -sum_j e_j * (r_j - log(num_j / den_j)) / sum_j e_j + 0.01 * ||W||_F

Strategy: shard columns j across the 8 cores (each core owns 2048 columns).
The N x 2048 mask is materialized on-chip in [128, 2048] tiles and contracted
on the TensorEngine against lhsT = [exp_hi, exp_lo, 1, 0...] (bf16 Dekker
split, padded to 32 rows) into PSUM.

Perf structure:
  * y is re-encoded on the host as monotone bf16 codes (rank -> bf16 bit
    pattern + 0x2000), so comparisons are exact in bf16 and the DVE
    tensor_scalar(is_le) compare runs in the 4x perf mode (~0.74us per
    [128, 2048] tile vs ~1.1us for the f32 compare).
  * Row-side code copies carry a +quarter-ulp offset so code_i' > code_j
    strictly for i == j: the ScalarE Sign producer yields exactly +/-1 and
    all tie/diagonal corrections vanish.  DVE produces 93 tiles, ACT 35
    (sign-encoded with halved weights; ~2.0us/tile), balancing the engines.
  * Matmuls are column-tiled 4 ways (tile_position=(0, 32g)): four thin-M
    matmuls execute concurrently in disjoint 32-column strips of the PE
    array.  PSUM group g accumulates i-tiles 32g..32g+31 at partitions
    32g..32g+31 (rows 3..31 zero-padded so the epilogue can read PSUM
    full-width).
  * The sign-encoding corrections (+V_half into hi/lo rows, +NACT/2 per
    partition into den) are folded in by one extra matmul per jj chunk
    against an all-ones rhs -- no scalar broadcast round-trip.
  * Epilogue: PSUM -> SBUF copies chunked across scalar+vector, a selector
    matmul folds the 4 groups' rows into [num | den], 2 contiguous-dest
    scatter DMAs redistribute to [128, 16] pf layout, and each core emits
    [e_sum, w_ssq, t_sum]; the host unshard sums t over cores and applies
    -t/e + 0.01*sqrt(w_ssq).
"""
import numpy as np
import ml_dtypes
import orjson

import concourse.bass as bass
import concourse.tile as tile
import concourse.mybir as mybir
from concourse.bass_utils import run_bass_kernel_spmd

F32 = mybir.dt.float32
BF16 = mybir.dt.bfloat16

N = 16384
NCORES = 8
JSHARD = N // NCORES            # 2048 columns per core
NT = N // 128                   # 128 i-tiles of 128 rows
NG = 4                          # PE column-strip groups
NR = NT // NG                   # 32 i-tiles (rounds) per group
NJJ = JSHARD // 512             # 4 matmul column chunks per core
NACT = 34                       # ACT-produced tiles (sign-encoded)
DEN_ROW = float(NACT) / 2.0     # per-partition den correction row


def tile_of(g, r):
    return 32 * g + r


def is_act(g, r):
    # ACT owns all of group 3 (its round-31 tile is ACT's last product, so
    # the PE finishes the final round right as ACT finishes -- an earlier
    # last-ACT-round would leave the PE draining a multi-round backlog at
    # ~0.86us per round after the producers are already done), plus two
    # mid-stream group-2 extras.
    return g == 3 or (g == 2 and r in (10, 21))


ACT_TILES = [tile_of(g, r) for g in range(NG) for r in range(NR) if is_act(g, r)]

# ---------------------------------------------------------------------------
# Workaround for the installed walrus accepting at most ONE sync-wait command
# per TPB instruction: split multi-wait instructions into preceding
# single-wait EventSemaphore instructions on the same engine.
# ---------------------------------------------------------------------------


def _fix_bir_multiwait(bir_json: bytes) -> bytes:
    d = orjson.loads(bir_json)
    counter = 0
    for fn in d.get("functions", []):
        stack = list(fn.get("blocks", []))
        while stack:
            block = stack.pop()
            stack.extend(block.get("blocks", []))
            new_insts = []
            for inst in block.get("instructions", []):
                sync = inst.get("sync_info") or {}
                waits = sync.get("on_wait") or []
                if len(waits) > 1:
                    for w in waits[:-1]:
                        counter += 1
                        new_insts.append({
                            "debug": inst.get("debug", 0),
                            "engine": inst.get("engine"),
                            "ins": [],
                            "name": f"esw_fix_{counter}",
                            "opcode": "EventSemaphore",
                            "outs": [],
                            "sync_info": {"on_update": [], "on_wait": [w]},
                        })
                    sync["on_wait"] = [waits[-1]]
                new_insts.append(inst)
            block["instructions"] = new_insts
    return orjson.dumps(d)


_patched = False


def _install_bir_fix():
    global _patched
    if _patched:
        return
    _patched = True
    import concourse.bass_utils as bu
    import concourse.bass2jax as b2j

    orig = bu.compile_bir_kernel

    def patched(bir_json, tmpdir, neff_name="file.neff"):
        if isinstance(bir_json, str):
            bir_json = bir_json.encode()
        return orig(_fix_bir_multiwait(bir_json), tmpdir, neff_name)

    bu.compile_bir_kernel = patched
    b2j.compile_bir_kernel = patched


# ---------------------------------------------------------------------------
# Kernel build
# ---------------------------------------------------------------------------

def build_kernel() -> bass.Bass:
    nc = bass.Bass()
    Sign = mybir.ActivationFunctionType.Sign

    # ONE wide critical load (6KB lines amortize the per-line DMA cost, and
    # a dma_start occupies its issuing engine for the whole transfer, so
    # fewer/wider is strictly better): [yb bf16 as f32-pairs | ycol | rcol |
    # scale_b | indh_b]
    ALLF_W = JSHARD // 2 + NT + NT + NT + NT
    allf = nc.dram_tensor("allf", [128, ALLF_W], F32, kind="ExternalInput")
    # rest: [r_pf | e_pf | e_f | w | sel] -- off the critical path, rides
    # the slow gpsimd software-dge queue in parallel
    REST_W = 16 + 16 + NT + 1024 + 2
    rest = nc.dram_tensor("rest", [128, REST_W], F32, kind="ExternalInput")
    out = nc.dram_tensor("out", [1, 3], F32, kind="ExternalOutput")

    with tile.TileContext(nc) as tc:
        with (
            tc.tile_pool(name="const", bufs=1) as const,
            tc.tile_pool(name="masks", bufs=26) as masks,
            tc.tile_pool(name="psacc", bufs=1, space="PSUM") as psacc,
            tc.tile_pool(name="psaux", bufs=1, space="PSUM") as psaux,
        ):
            # ---- DVE-local init first (no input deps; overlaps the DMAs)
            ones_col = const.tile([128, 1], F32)
            nc.vector.memset(ones_col, 1.0)
            zeros_32 = const.tile([128, 32], BF16)
            nc.vector.memset(zeros_32, 0.0)
            ones_b = const.tile([128, 512], BF16)
            nc.vector.memset(ones_b, 1.0)
            lhsT = const.tile([128, 3, NT], BF16)

            # ---- critical-path loads: ONE wide DMA on sync (a dma_start
            # occupies its issuing engine for the whole transfer, and wide
            # 6KB lines amortize the per-line cost); rest rides the slow
            # gpsimd software-dge queue fully in parallel
            allf_sb = const.tile([128, ALLF_W], F32)
            nc.sync.dma_start(out=allf_sb, in_=allf[:, :])
            rest_sb = const.tile([128, REST_W], F32)
            nc.gpsimd.dma_start(out=rest_sb, in_=rest[:, :])
            o = JSHARD // 2
            yb = allf_sb[:, 0:o].bitcast(BF16)         # j-side codes
            ycol_sb = allf_sb[:, o:o + NT]; o += NT    # code_i + delta (f32)
            rcol_sb = allf_sb[:, o:o + NT]; o += NT    # risk_pred col-major
            scale_b = allf_sb[:, o:o + NT]; o += NT    # 0.5 on ACT cols else 1
            indh_b = allf_sb[:, o:o + NT]; o += NT     # 0.5 on ACT cols else 0
            o = 0
            rpf_sb = rest_sb[:, o:o + 16]; o += 16
            epf_f = rest_sb[:, o:o + 16]; o += 16
            e_f = rest_sb[:, o:o + NT]; o += NT
            w_sb = rest_sb[:, o:o + 1024]; o += 1024
            sel_sb = rest_sb[:, o:o + 2]; o += 2       # group-fold selectors

            exp_sb = const.tile([128, NT], F32)
            nc.scalar.activation(exp_sb, rcol_sb, mybir.ActivationFunctionType.Exp)

            # ---- main loop: mask tiles + column-tiled matmul accumulation
            # acc group g lives at partitions [32g, 32g+32), banks by jj chunk
            acc = psacc.tile([128, NJJ * 512], F32)

            # start-of-chain init matmuls: zero weights, start=True.  They
            # zero ALL 32 partitions of each group's PSUM block and set
            # has_written, so the thin [3, 512] main matmuls can accumulate
            # with start=False and the epilogue can read PSUM full-width.
            # (Also serves as the PE HAM warm-up.)
            for g in range(NG):
                for jj in range(NJJ):
                    nc.tensor.matmul(
                        acc[32 * g:32 * g + 32, 512 * jj:512 * (jj + 1)],
                        zeros_32, ones_b,
                        start=True, stop=False,
                        tile_position=(0, 32 * g), skip_group_check=True,
                    )

            lhsT_va = const.tile([128, 32], BF16)
            vh = const.tile([128, NT], F32)
            vred = const.tile([128, 1], F32)
            vhi32 = const.tile([128, 1], F32)
            vlo = const.tile([128, 1], F32)
            hi32 = const.tile([128, NT], F32)
            lo32 = const.tile([128, NT], F32)
            vec3 = const.tile([128, 3], F32)

            for r in range(NR):
                mt = {}
                for g in range(NG):
                    t = tile_of(g, r)
                    m = masks.tile([128, JSHARD], BF16)
                    mt[g] = m
                    if is_act(g, r):
                        nc.scalar.activation(
                            m, yb, Sign, bias=ycol_sb[:, t:t + 1], scale=-1.0,
                        )
                    else:
                        nc.vector.tensor_scalar(
                            out=m, in0=yb,
                            scalar1=ycol_sb[:, t:t + 1], scalar2=None,
                            op0=mybir.AluOpType.is_le,
                        )
                if r == 0:
                    # lhsT rows 0..2 = scale * [exp_hi | exp_lo | ones], bf16
                    # (emitted after the first masks so they head the DVE
                    # queue, but before any matmul reads lhsT)
                    nc.vector.tensor_copy(lhsT[:, 0, :], exp_sb)   # bf16(exp)
                    nc.vector.tensor_copy(hi32, lhsT[:, 0, :])     # back to f32
                    nc.vector.tensor_sub(lo32, exp_sb, hi32)       # f32 residual
                    nc.vector.tensor_mul(lhsT[:, 0, :], hi32, scale_b)
                    nc.vector.tensor_mul(lhsT[:, 1, :], lo32, scale_b)
                    nc.vector.tensor_copy(lhsT[:, 2, :], scale_b)
                for g in range(NG):
                    t = tile_of(g, r)
                    for jj in range(NJJ):
                        nc.tensor.matmul(
                            acc[32 * g:32 * g + 3, 512 * jj:512 * (jj + 1)],
                            lhsT[:, :, t],
                            mt[g][:, 512 * jj:512 * (jj + 1)],
                            start=False,
                            stop=(r == NR - 1),
                            tile_position=(0, 32 * g),
                            skip_group_check=True,
                        )
                # deferred off-critical DVE prologue work (keeps the first
                # mask tiles at the head of the DVE queue)
                if r == 2:
                    nc.vector.memset(lhsT_va, 0.0)
                    nc.vector.tensor_mul(vh, exp_sb, indh_b)
                    nc.vector.tensor_reduce(
                        out=vred, in_=vh, axis=mybir.AxisListType.X,
                        op=mybir.AluOpType.add)
                elif r == 3:
                    nc.vector.tensor_copy(lhsT_va[:, 0:1], vred)  # vred_hi
                    nc.vector.tensor_copy(vhi32, lhsT_va[:, 0:1])
                    nc.vector.tensor_sub(vlo, vred, vhi32)
                    nc.vector.tensor_copy(lhsT_va[:, 1:2], vlo)   # vred_lo
                    nc.vector.memset(lhsT_va[:, 2:3], DEN_ROW)
                elif r == 8:
                    nc.vector.tensor_reduce(
                        out=vec3[:, 0:1], in_=e_f, axis=mybir.AxisListType.X,
                        op=mybir.AluOpType.add)
                elif r == 6:
                    # correction matmuls, mid-stream: add [V_half_hi,
                    # V_half_lo, NACT/2] (summed over partitions by the PE)
                    # into the group-0 rows for every column
                    for jj in range(NJJ):
                        nc.tensor.matmul(
                            acc[0:32, 512 * jj:512 * (jj + 1)],
                            lhsT_va, ones_b,
                            start=False, stop=False,
                            tile_position=(0, 0), skip_group_check=True,
                        )

            # ---- late ACT-side work that overlaps the epilogue
            w2d = const.tile([128, 1024], F32)
            nc.scalar.activation(
                w2d, w_sb, mybir.ActivationFunctionType.Square,
                accum_out=vec3[:, 1:2],
            )

            # ---- epilogue: PSUM -> SBUF staging (chunked, both engines),
            # selector matmul folds 4 groups' (hi+lo) and den rows into
            # [num | den], 2 contiguous-dest scatter DMAs into pf layout.
            # pf mapping: x_pf[p, c] = x_shard[16*p + c]
            F32R = mybir.dt.float32r
            sel_r = const.tile([128, 2], F32R)
            nc.vector.tensor_copy(sel_r, sel_sb)
            nd_all = const.tile([128, NJJ * 512], F32R)
            fold = psaux.tile([2, NJJ * 512], F32, name="fold")
            lnnd = const.tile([2, NJJ * 512], F32)
            lnn = const.tile([128, 16], F32)
            lnd = const.tile([128, 16], F32)
            for h in range(2):
                cs = slice(1024 * h, 1024 * (h + 1))
                if h == 0:
                    nc.scalar.copy(nd_all[:, cs], acc[:, cs])
                else:
                    nc.vector.tensor_copy(nd_all[:, cs], acc[:, cs])
                # f32r: single-pass reduced-precision fp32 matmul (fp22
                # mantissa -- plenty for the 2e-2 budget, 2x faster)
                for jj in (2 * h, 2 * h + 1):
                    nc.tensor.matmul(
                        fold[:, 512 * jj:512 * (jj + 1)],
                        sel_r, nd_all[:, 512 * jj:512 * (jj + 1)],
                        start=True, stop=True, skip_group_check=True,
                    )
                # Ln straight off the PSUM fold rows (both rows per chunk),
                # then scatter this half into pf partitions [64h, 64h+64)
                nc.scalar.activation(lnnd[:, cs], fold[:, cs],
                                     mybir.ActivationFunctionType.Ln)
                nc.sync.dma_start(out=lnn[64 * h:64 * (h + 1), :],
                                  in_=lnnd[0:1, cs])
                nc.scalar.dma_start(out=lnd[64 * h:64 * (h + 1), :],
                                    in_=lnnd[1:2, cs])

            # ---- wide final math on [128, 16]
            s1 = const.tile([128, 16], F32)
            nc.vector.tensor_sub(s1, rpf_sb, lnn[:, :])
            s2 = const.tile([128, 16], F32)
            nc.vector.scalar_tensor_tensor(
                out=s2, in0=s1, scalar=1.0, in1=lnd,
                op0=mybir.AluOpType.mult, op1=mybir.AluOpType.add)
            s3 = const.tile([128, 16], F32)
            nc.vector.scalar_tensor_tensor(
                out=s3, in0=s2, scalar=1.0, in1=epf_f,
                op0=mybir.AluOpType.mult, op1=mybir.AluOpType.mult,
                accum_out=vec3[:, 2:3])

            # ---- cross-partition fold: [e_sum, w_ssq, t_sum] into one row
            # (reuses a slice of the fold tile; WAR deps order it after nd2)
            sums = fold[0:1, 0:3]
            nc.tensor.matmul(sums, ones_col, vec3[:, :], start=True,
                             stop=True, skip_group_check=True)
            res3 = const.tile([1, 3], F32)
            nc.scalar.copy(res3, sums)
            nc.scalar.dma_start(out=out[:, :], in_=res3)

    return nc


_nc_cache = None


def _get_nc():
    global _nc_cache
    if _nc_cache is None:
        _install_bir_fix()
        _nc_cache = build_kernel()
    return _nc_cache


def make_in_maps(risk_pred, y, e, W):
    """Host-side sharding: slice/reshape/encode the full inputs per core."""
    yflat = y.reshape(-1)
    # monotone distinct bf16 codes: rank -> bf16 bit pattern (+0x2000 keeps
    # every code and its successor a normal number in [2^-63, 2^64], so
    # all pairwise differences are far from f32 under/overflow)
    order = np.argsort(yflat, kind="stable")
    ranks = np.empty(N, np.uint16)
    ranks[order] = np.arange(N, dtype=np.uint16)
    codes_u16 = (ranks + np.uint16(0x2000)).astype(np.uint16)
    codes_bf16 = codes_u16.view(ml_dtypes.bfloat16)
    codes_f32 = codes_bf16.astype(np.float32)
    nxt_f32 = (codes_u16 + np.uint16(1)).view(ml_dtypes.bfloat16).astype(np.float32)
    # row-side codes get +quarter-gap so the diagonal compare is strict (+1)
    ycol_delta = codes_f32 + 0.25 * (nxt_f32 - codes_f32)

    ycol = ycol_delta.reshape(NT, 128).T                     # [p, t]
    rcol = risk_pred.reshape(NT, 128).T.astype(np.float32)
    ef = e.astype(np.float32).reshape(NT, 128).T
    w_flat = W.reshape(128, 1024).astype(np.float32)
    act_mask = np.zeros(NT, np.float32)
    act_mask[ACT_TILES] = 1.0
    scale_b = np.tile(1.0 - 0.5 * act_mask, (128, 1)).astype(np.float32)
    indh_b = np.tile(0.5 * act_mask, (128, 1)).astype(np.float32)
    # group-fold selector: col 0 sums the hi+lo rows (p%32 in {0,1}) of the
    # 4 PSUM groups, col 1 sums the den rows (p%32 == 2)
    p = np.arange(128)
    sel = np.stack([((p % 32) <= 1), ((p % 32) == 2)], axis=1).astype(np.float32)

    crit = np.ascontiguousarray(np.concatenate(
        [ycol, rcol, scale_b, indh_b], axis=1), dtype=np.float32)

    in_maps = []
    for c in range(NCORES):
        j0 = c * JSHARD
        rsh = risk_pred.reshape(-1)[j0:j0 + JSHARD]
        esh = e.astype(np.float32).reshape(-1)[j0:j0 + JSHARD]
        r_pf = rsh.reshape(128, 16).astype(np.float32)
        e_pf = esh.reshape(128, 16)
        rest = np.ascontiguousarray(np.concatenate(
            [r_pf, e_pf, ef, w_flat, sel], axis=1), dtype=np.float32)
        yb = np.ascontiguousarray(
            np.broadcast_to(codes_bf16[j0:j0 + JSHARD], (128, JSHARD)))
        allf = np.ascontiguousarray(np.concatenate(
            [yb.view(np.float32), crit], axis=1), dtype=np.float32)
        in_maps.append(dict(allf=allf, rest=rest))
    return in_maps


def kernel(risk_pred, y, e, W, **run_kwargs):
    nc = _get_nc()
    in_maps = make_in_maps(
        np.asarray(risk_pred, np.float32),
        np.asarray(y, np.float32),
        np.asarray(e, np.int32),
        np.asarray(W, np.float32),
    )
    result = run_bass_kernel_spmd(nc, in_maps, core_ids=list(range(NCORES)),
                                  **run_kwargs)
    # gather/unshard: t_sum adds across cores; e_sum and w_ssq are computed
    # from replicated inputs (identical on every core)
    t_total = np.float32(0.0)
    for r in result.results:
        t_total = np.float32(t_total + r["out"][0, 2])
    e_sum = np.float32(result.results[0]["out"][0, 0])
    w_ssq = np.float32(result.results[0]["out"][0, 1])
    total = np.float32(-t_total / e_sum + np.float32(0.01) * np.sqrt(w_ssq))
    kernel.last_result = result
    return np.asarray(total, np.float32)


# revision 71
# speedup vs baseline: 1.0808x; 1.0018x over previous
"""Cox partial-likelihood NegativeLogLikelihood loss on 8 Trainium2 cores.

reference:
    mask[i, j] = (y[j] <= y[i])                       # (N, N)
    num[j] = sum_i exp(r_i) * mask[i, j]
    den[j] = sum_i mask[i, j]
    loss = -sum_j e_j * (r_j - log(num_j / den_j)) / sum_j e_j + 0.01 * ||W||_F

Strategy: shard columns j across the 8 cores (each core owns 2048 columns).
The N x 2048 mask is materialized on-chip in [128, 2048] tiles and contracted
on the TensorEngine against lhsT = [exp_hi, exp_lo, 1, 0...] (bf16 Dekker
split, padded to 32 rows) into PSUM.

Perf structure:
  * y is re-encoded on the host as monotone bf16 codes (rank -> bf16 bit
    pattern + 0x2000), so comparisons are exact in bf16 and the DVE
    tensor_scalar(is_le) compare runs in the 4x perf mode (~0.74us per
    [128, 2048] tile vs ~1.1us for the f32 compare).
  * Row-side code copies carry a +quarter-ulp offset so code_i' > code_j
    strictly for i == j: the ScalarE Sign producer yields exactly +/-1 and
    all tie/diagonal corrections vanish.  DVE produces 93 tiles, ACT 35
    (sign-encoded with halved weights; ~2.0us/tile), balancing the engines.
  * Matmuls are column-tiled 4 ways (tile_position=(0, 32g)): four thin-M
    matmuls execute concurrently in disjoint 32-column strips of the PE
    array.  PSUM group g accumulates i-tiles 32g..32g+31 at partitions
    32g..32g+31 (rows 3..31 zero-padded so the epilogue can read PSUM
    full-width).
  * The sign-encoding corrections (+V_half into hi/lo rows, +NACT/2 per
    partition into den) are folded in by one extra matmul per jj chunk
    against an all-ones rhs -- no scalar broadcast round-trip.
  * Epilogue: PSUM -> SBUF copies chunked across scalar+vector, a selector
    matmul folds the 4 groups' rows into [num | den], 2 contiguous-dest
    scatter DMAs redistribute to [128, 16] pf layout, and each core emits
    [e_sum, w_ssq, t_sum]; the host unshard sums t over cores and applies
    -t/e + 0.01*sqrt(w_ssq).
"""
import numpy as np
import ml_dtypes
import orjson

import concourse.bass as bass
import concourse.tile as tile
import concourse.mybir as mybir
from concourse.bass_utils import run_bass_kernel_spmd

F32 = mybir.dt.float32
BF16 = mybir.dt.bfloat16

N = 16384
NCORES = 8
JSHARD = N // NCORES            # 2048 columns per core
NT = N // 128                   # 128 i-tiles of 128 rows
NG = 4                          # PE column-strip groups
NR = NT // NG                   # 32 i-tiles (rounds) per group
NJJ = JSHARD // 512             # 4 matmul column chunks per core
NACT = 35                       # ACT-produced tiles (sign-encoded)
DEN_ROW = float(NACT) / 2.0     # per-partition den correction row


def tile_of(g, r):
    return 32 * g + r


def is_act(g, r):
    # ACT owns all of group 3 (its round-31 tile is ACT's last product, so
    # the PE finishes the final round right as ACT finishes -- an earlier
    # last-ACT-round would leave the PE draining a multi-round backlog at
    # ~0.86us per round after the producers are already done), plus two
    # mid-stream group-2 extras.
    return g == 3 or (g == 2 and r in (8, 16, 24))


ACT_TILES = [tile_of(g, r) for g in range(NG) for r in range(NR) if is_act(g, r)]

# ---------------------------------------------------------------------------
# Workaround for the installed walrus accepting at most ONE sync-wait command
# per TPB instruction: split multi-wait instructions into preceding
# single-wait EventSemaphore instructions on the same engine.
# ---------------------------------------------------------------------------


def _fix_bir_multiwait(bir_json: bytes) -> bytes:
    d = orjson.loads(bir_json)
    counter = 0
    for fn in d.get("functions", []):
        stack = list(fn.get("blocks", []))
        while stack:
            block = stack.pop()
            stack.extend(block.get("blocks", []))
            new_insts = []
            for inst in block.get("instructions", []):
                sync = inst.get("sync_info") or {}
                waits = sync.get("on_wait") or []
                if len(waits) > 1:
                    for w in waits[:-1]:
                        counter += 1
                        new_insts.append({
                            "debug": inst.get("debug", 0),
                            "engine": inst.get("engine"),
                            "ins": [],
                            "name": f"esw_fix_{counter}",
                            "opcode": "EventSemaphore",
                            "outs": [],
                            "sync_info": {"on_update": [], "on_wait": [w]},
                        })
                    sync["on_wait"] = [waits[-1]]
                new_insts.append(inst)
            block["instructions"] = new_insts
    return orjson.dumps(d)


_patched = False


def _install_bir_fix():
    global _patched
    if _patched:
        return
    _patched = True
    import concourse.bass_utils as bu
    import concourse.bass2jax as b2j

    orig = bu.compile_bir_kernel

    def patched(bir_json, tmpdir, neff_name="file.neff"):
        if isinstance(bir_json, str):
            bir_json = bir_json.encode()
        return orig(_fix_bir_multiwait(bir_json), tmpdir, neff_name)

    bu.compile_bir_kernel = patched
    b2j.compile_bir_kernel = patched


# ---------------------------------------------------------------------------
# Kernel build
# ---------------------------------------------------------------------------

def build_kernel() -> bass.Bass:
    nc = bass.Bass()
    Sign = mybir.ActivationFunctionType.Sign

    # ONE wide critical load (6KB lines amortize the per-line DMA cost, and
    # a dma_start occupies its issuing engine for the whole transfer, so
    # fewer/wider is strictly better): [yb bf16 as f32-pairs | ycol | rcol |
    # scale_b | indh_b]
    ALLF_W = JSHARD // 2 + NT + NT + NT + NT
    allf = nc.dram_tensor("allf", [128, ALLF_W], F32, kind="ExternalInput")
    # rest: [r_pf | e_pf | e_f | w | sel] -- off the critical path, rides
    # the slow gpsimd software-dge queue in parallel
    REST_W = 16 + 16 + NT + 1024 + 2
    rest = nc.dram_tensor("rest", [128, REST_W], F32, kind="ExternalInput")
    out = nc.dram_tensor("out", [1, 3], F32, kind="ExternalOutput")

    with tile.TileContext(nc) as tc:
        with (
            tc.tile_pool(name="const", bufs=1) as const,
            tc.tile_pool(name="masks", bufs=26) as masks,
            tc.tile_pool(name="psacc", bufs=1, space="PSUM") as psacc,
            tc.tile_pool(name="psaux", bufs=1, space="PSUM") as psaux,
        ):
            # ---- DVE-local init first (no input deps; overlaps the DMAs)
            ones_col = const.tile([128, 1], F32)
            nc.vector.memset(ones_col, 1.0)
            zeros_32 = const.tile([128, 32], BF16)
            nc.vector.memset(zeros_32, 0.0)
            ones_b = const.tile([128, 512], BF16)
            nc.vector.memset(ones_b, 1.0)
            lhsT = const.tile([128, 3, NT], BF16)

            # ---- critical-path loads: the wide critical pack is split
            # across BOTH hw-dge queues so the two ~3us transfers run in
            # parallel (a dma_start occupies its issuing engine for the
            # whole transfer); rest rides the slow gpsimd software-dge
            # queue fully in parallel
            allf_sb = const.tile([128, ALLF_W], F32)
            nc.sync.dma_start(out=allf_sb[:, 0:896], in_=allf[:, 0:896])
            nc.scalar.dma_start(out=allf_sb[:, 896:ALLF_W],
                                in_=allf[:, 896:ALLF_W])
            rest_sb = const.tile([128, REST_W], F32)
            nc.gpsimd.dma_start(out=rest_sb, in_=rest[:, :])
            o = JSHARD // 2
            yb = allf_sb[:, 0:o].bitcast(BF16)         # j-side codes
            ycol_sb = allf_sb[:, o:o + NT]; o += NT    # code_i + delta (f32)
            rcol_sb = allf_sb[:, o:o + NT]; o += NT    # risk_pred col-major
            scale_b = allf_sb[:, o:o + NT]; o += NT    # 0.5 on ACT cols else 1
            indh_b = allf_sb[:, o:o + NT]; o += NT     # 0.5 on ACT cols else 0
            o = 0
            rpf_sb = rest_sb[:, o:o + 16]; o += 16
            epf_f = rest_sb[:, o:o + 16]; o += 16
            e_f = rest_sb[:, o:o + NT]; o += NT
            w_sb = rest_sb[:, o:o + 1024]; o += 1024
            sel_sb = rest_sb[:, o:o + 2]; o += 2       # group-fold selectors

            exp_sb = const.tile([128, NT], F32)
            nc.scalar.activation(exp_sb, rcol_sb, mybir.ActivationFunctionType.Exp)

            # ---- main loop: mask tiles + column-tiled matmul accumulation
            # acc group g lives at partitions [32g, 32g+32), banks by jj chunk
            acc = psacc.tile([128, NJJ * 512], F32)

            # start-of-chain init matmuls: zero weights, start=True.  They
            # zero ALL 32 partitions of each group's PSUM block and set
            # has_written, so the thin [3, 512] main matmuls can accumulate
            # with start=False and the epilogue can read PSUM full-width.
            # (Also serves as the PE HAM warm-up.)
            for g in range(NG):
                for jj in range(NJJ):
                    nc.tensor.matmul(
                        acc[32 * g:32 * g + 32, 512 * jj:512 * (jj + 1)],
                        zeros_32, ones_b,
                        start=True, stop=False,
                        tile_position=(0, 32 * g), skip_group_check=True,
                    )

            lhsT_va = const.tile([128, 32], BF16)
            vh = const.tile([128, NT], F32)
            vred = const.tile([128, 1], F32)
            vhi32 = const.tile([128, 1], F32)
            vlo = const.tile([128, 1], F32)
            hi32 = const.tile([128, NT], F32)
            lo32 = const.tile([128, NT], F32)
            vec3 = const.tile([128, 3], F32)

            for r in range(NR):
                mt = {}
                for g in range(NG):
                    t = tile_of(g, r)
                    m = masks.tile([128, JSHARD], BF16)
                    mt[g] = m
                    if is_act(g, r):
                        nc.scalar.activation(
                            m, yb, Sign, bias=ycol_sb[:, t:t + 1], scale=-1.0,
                        )
                    else:
                        nc.vector.tensor_scalar(
                            out=m, in0=yb,
                            scalar1=ycol_sb[:, t:t + 1], scalar2=None,
                            op0=mybir.AluOpType.is_le,
                        )
                if r == 0:
                    # lhsT rows 0..2 = scale * [exp_hi | exp_lo | ones], bf16
                    # (emitted after the first masks so they head the DVE
                    # queue, but before any matmul reads lhsT)
                    nc.vector.tensor_copy(lhsT[:, 0, :], exp_sb)   # bf16(exp)
                    nc.vector.tensor_copy(hi32, lhsT[:, 0, :])     # back to f32
                    nc.vector.tensor_sub(lo32, exp_sb, hi32)       # f32 residual
                    nc.vector.tensor_mul(lhsT[:, 0, :], hi32, scale_b)
                    nc.vector.tensor_mul(lhsT[:, 1, :], lo32, scale_b)
                    nc.vector.tensor_copy(lhsT[:, 2, :], scale_b)
                for g in range(NG):
                    t = tile_of(g, r)
                    for jj in range(NJJ):
                        nc.tensor.matmul(
                            acc[32 * g:32 * g + 3, 512 * jj:512 * (jj + 1)],
                            lhsT[:, :, t],
                            mt[g][:, 512 * jj:512 * (jj + 1)],
                            start=False,
                            stop=(r == NR - 1),
                            tile_position=(0, 32 * g),
                            skip_group_check=True,
                        )
                # deferred off-critical DVE prologue work (keeps the first
                # mask tiles at the head of the DVE queue)
                if r == 2:
                    nc.vector.memset(lhsT_va, 0.0)
                    nc.vector.tensor_mul(vh, exp_sb, indh_b)
                    nc.vector.tensor_reduce(
                        out=vred, in_=vh, axis=mybir.AxisListType.X,
                        op=mybir.AluOpType.add)
                elif r == 3:
                    nc.vector.tensor_copy(lhsT_va[:, 0:1], vred)  # vred_hi
                    nc.vector.tensor_copy(vhi32, lhsT_va[:, 0:1])
                    nc.vector.tensor_sub(vlo, vred, vhi32)
                    nc.vector.tensor_copy(lhsT_va[:, 1:2], vlo)   # vred_lo
                    nc.vector.memset(lhsT_va[:, 2:3], DEN_ROW)
                elif r == 8:
                    nc.vector.tensor_reduce(
                        out=vec3[:, 0:1], in_=e_f, axis=mybir.AxisListType.X,
                        op=mybir.AluOpType.add)
                elif r == 6:
                    # correction matmuls, mid-stream: add [V_half_hi,
                    # V_half_lo, NACT/2] (summed over partitions by the PE)
                    # into the group-0 rows for every column
                    for jj in range(NJJ):
                        nc.tensor.matmul(
                            acc[0:32, 512 * jj:512 * (jj + 1)],
                            lhsT_va, ones_b,
                            start=False, stop=False,
                            tile_position=(0, 0), skip_group_check=True,
                        )

            # ---- epilogue: PSUM -> SBUF staging (chunked, both engines),
            # selector matmul folds 4 groups' (hi+lo) and den rows into
            # [num | den], 2 contiguous-dest scatter DMAs into pf layout.
            # pf mapping: x_pf[p, c] = x_shard[16*p + c]
            F32R = mybir.dt.float32r
            sel_r = const.tile([128, 2], F32R)
            nc.vector.tensor_copy(sel_r, sel_sb)
            nd_all = const.tile([128, NJJ * 512], F32R)
            fold = psaux.tile([2, NJJ * 512], F32, name="fold")
            lnnd = const.tile([2, NJJ * 512], F32)
            lnn = const.tile([128, 16], F32)
            lnd = const.tile([128, 16], F32)
            s1 = const.tile([128, 16], F32)
            s2 = const.tile([128, 16], F32)
            s3 = const.tile([128, 16], F32)
            for h in range(2):
                cs = slice(1024 * h, 1024 * (h + 1))
                if h == 0:
                    nc.scalar.copy(nd_all[:, cs], acc[:, cs])
                else:
                    nc.vector.tensor_copy(nd_all[:, cs], acc[:, cs])
                # f32r: single-pass reduced-precision fp32 matmul (fp22
                # mantissa -- plenty for the 2e-2 budget, 2x faster)
                for jj in (2 * h, 2 * h + 1):
                    nc.tensor.matmul(
                        fold[:, 512 * jj:512 * (jj + 1)],
                        sel_r, nd_all[:, 512 * jj:512 * (jj + 1)],
                        start=True, stop=True, skip_group_check=True,
                    )
                # Ln straight off the PSUM fold rows (both rows per chunk),
                # then scatter this half into pf partitions [64h, 64h+64)
                # (both scatters on the otherwise-idle sync queue so the
                # scalar engine proceeds straight to the next Ln), and run
                # the [64, 16] final math for this half immediately
                nc.scalar.activation(lnnd[:, cs], fold[:, cs],
                                     mybir.ActivationFunctionType.Ln)
                ps = slice(64 * h, 64 * (h + 1))
                nc.sync.dma_start(out=lnn[ps, :], in_=lnnd[0:1, cs])
                nc.sync.dma_start(out=lnd[ps, :], in_=lnnd[1:2, cs])
                nc.vector.tensor_sub(s1[ps, :], rpf_sb[ps, :], lnn[ps, :])
                nc.vector.scalar_tensor_tensor(
                    out=s2[ps, :], in0=s1[ps, :], scalar=1.0, in1=lnd[ps, :],
                    op0=mybir.AluOpType.mult, op1=mybir.AluOpType.add)
                nc.vector.scalar_tensor_tensor(
                    out=s3[ps, :], in0=s2[ps, :], scalar=1.0,
                    in1=epf_f[ps, :],
                    op0=mybir.AluOpType.mult, op1=mybir.AluOpType.mult,
                    accum_out=vec3[ps, 2:3])

            # ---- W frobenius^2 (gates only the final sums matmul)
            w2d = const.tile([128, 1024], F32)
            nc.scalar.activation(
                w2d, w_sb, mybir.ActivationFunctionType.Square,
                accum_out=vec3[:, 1:2],
            )

            # ---- cross-partition fold: [e_sum, w_ssq, t_sum] into one row
            # (reuses a slice of the fold tile; WAR deps order it after nd2)
            sums = fold[0:1, 0:3]
            nc.tensor.matmul(sums, ones_col, vec3[:, :], start=True,
                             stop=True, skip_group_check=True)
            res3 = const.tile([1, 3], F32)
            nc.scalar.copy(res3, sums)
            nc.scalar.dma_start(out=out[:, :], in_=res3)

    return nc


_nc_cache = None


def _get_nc():
    global _nc_cache
    if _nc_cache is None:
        _install_bir_fix()
        _nc_cache = build_kernel()
    return _nc_cache


def make_in_maps(risk_pred, y, e, W):
    """Host-side sharding: slice/reshape/encode the full inputs per core."""
    yflat = y.reshape(-1)
    # monotone distinct bf16 codes: rank -> bf16 bit pattern (+0x2000 keeps
    # every code and its successor a normal number in [2^-63, 2^64], so
    # all pairwise differences are far from f32 under/overflow)
    order = np.argsort(yflat, kind="stable")
    ranks = np.empty(N, np.uint16)
    ranks[order] = np.arange(N, dtype=np.uint16)
    codes_u16 = (ranks + np.uint16(0x2000)).astype(np.uint16)
    codes_bf16 = codes_u16.view(ml_dtypes.bfloat16)
    codes_f32 = codes_bf16.astype(np.float32)
    nxt_f32 = (codes_u16 + np.uint16(1)).view(ml_dtypes.bfloat16).astype(np.float32)
    # row-side codes get +quarter-gap so the diagonal compare is strict (+1)
    ycol_delta = codes_f32 + 0.25 * (nxt_f32 - codes_f32)

    ycol = ycol_delta.reshape(NT, 128).T                     # [p, t]
    rcol = risk_pred.reshape(NT, 128).T.astype(np.float32)
    ef = e.astype(np.float32).reshape(NT, 128).T
    w_flat = W.reshape(128, 1024).astype(np.float32)
    act_mask = np.zeros(NT, np.float32)
    act_mask[ACT_TILES] = 1.0
    scale_b = np.tile(1.0 - 0.5 * act_mask, (128, 1)).astype(np.float32)
    indh_b = np.tile(0.5 * act_mask, (128, 1)).astype(np.float32)
    # group-fold selector: col 0 sums the hi+lo rows (p%32 in {0,1}) of the
    # 4 PSUM groups, col 1 sums the den rows (p%32 == 2)
    p = np.arange(128)
    sel = np.stack([((p % 32) <= 1), ((p % 32) == 2)], axis=1).astype(np.float32)

    crit = np.ascontiguousarray(np.concatenate(
        [ycol, rcol, scale_b, indh_b], axis=1), dtype=np.float32)

    in_maps = []
    for c in range(NCORES):
        j0 = c * JSHARD
        rsh = risk_pred.reshape(-1)[j0:j0 + JSHARD]
        esh = e.astype(np.float32).reshape(-1)[j0:j0 + JSHARD]
        r_pf = rsh.reshape(128, 16).astype(np.float32)
        e_pf = esh.reshape(128, 16)
        rest = np.ascontiguousarray(np.concatenate(
            [r_pf, e_pf, ef, w_flat, sel], axis=1), dtype=np.float32)
        yb = np.ascontiguousarray(
            np.broadcast_to(codes_bf16[j0:j0 + JSHARD], (128, JSHARD)))
        allf = np.ascontiguousarray(np.concatenate(
            [yb.view(np.float32), crit], axis=1), dtype=np.float32)
        in_maps.append(dict(allf=allf, rest=rest))
    return in_maps


def kernel(risk_pred, y, e, W, **run_kwargs):
    nc = _get_nc()
    in_maps = make_in_maps(
        np.asarray(risk_pred, np.float32),
        np.asarray(y, np.float32),
        np.asarray(e, np.int32),
        np.asarray(W, np.float32),
    )
    result = run_bass_kernel_spmd(nc, in_maps, core_ids=list(range(NCORES)),
                                  **run_kwargs)
    # gather/unshard: t_sum adds across cores; e_sum and w_ssq are computed
    # from replicated inputs (identical on every core)
    t_total = np.float32(0.0)
    for r in result.results:
        t_total = np.float32(t_total + r["out"][0, 2])
    e_sum = np.float32(result.results[0]["out"][0, 0])
    w_ssq = np.float32(result.results[0]["out"][0, 1])
    total = np.float32(-t_total / e_sum + np.float32(0.01) * np.sqrt(w_ssq))
    kernel.last_result = result
    return np.asarray(total, np.float32)


# revision 76
# speedup vs baseline: 1.0934x; 1.0116x over previous
"""Cox partial-likelihood NegativeLogLikelihood loss on 8 Trainium2 cores.

reference:
    mask[i, j] = (y[j] <= y[i])                       # (N, N)
    num[j] = sum_i exp(r_i) * mask[i, j]
    den[j] = sum_i mask[i, j]
    loss = -sum_j e_j * (r_j - log(num_j / den_j)) / sum_j e_j + 0.01 * ||W||_F

Strategy: shard columns j across the 8 cores (each core owns 2048 columns).
The N x 2048 mask is materialized on-chip in [128, 2048] tiles and contracted
on the TensorEngine against lhsT = [exp_hi, exp_lo, 1, 0...] (bf16 Dekker
split, padded to 32 rows) into PSUM.

Perf structure:
  * y is re-encoded on the host as monotone bf16 codes (rank -> bf16 bit
    pattern + 0x2000), so comparisons are exact in bf16 and the DVE
    tensor_scalar(is_le) compare runs in the 4x perf mode (~0.74us per
    [128, 2048] tile vs ~1.1us for the f32 compare).
  * Row-side code copies carry a +quarter-ulp offset so code_i' > code_j
    strictly for i == j: the ScalarE Sign producer yields exactly +/-1 and
    all tie/diagonal corrections vanish.  DVE produces 93 tiles, ACT 35
    (sign-encoded with halved weights; ~2.0us/tile), balancing the engines.
  * Matmuls are column-tiled 4 ways (tile_position=(0, 32g)): four thin-M
    matmuls execute concurrently in disjoint 32-column strips of the PE
    array.  PSUM group g accumulates i-tiles 32g..32g+31 at partitions
    32g..32g+31 (rows 3..31 zero-padded so the epilogue can read PSUM
    full-width).
  * The sign-encoding corrections (+V_half into hi/lo rows, +NACT/2 per
    partition into den) are folded in by one extra matmul per jj chunk
    against an all-ones rhs -- no scalar broadcast round-trip.
  * Epilogue: PSUM -> SBUF copies chunked across scalar+vector, a selector
    matmul folds the 4 groups' rows into [num | den], 2 contiguous-dest
    scatter DMAs redistribute to [128, 16] pf layout, and each core emits
    [e_sum, w_ssq, t_sum]; the host unshard sums t over cores and applies
    -t/e + 0.01*sqrt(w_ssq).
"""
import numpy as np
import ml_dtypes
import orjson

import concourse.bass as bass
import concourse.tile as tile
import concourse.mybir as mybir
from concourse.bass_utils import run_bass_kernel_spmd

F32 = mybir.dt.float32
BF16 = mybir.dt.bfloat16

N = 16384
NCORES = 8
JSHARD = N // NCORES            # 2048 columns per core
NT = N // 128                   # 128 i-tiles of 128 rows
NG = 4                          # PE column-strip groups
NR = NT // NG                   # 32 i-tiles (rounds) per group
NJJ = JSHARD // 512             # 4 matmul column chunks per core
NACT = 34                       # ACT-produced tiles (sign-encoded)
DEN_ROW = float(NACT) / 2.0     # per-partition den correction row


def tile_of(g, r):
    return 32 * g + r


def is_act(g, r):
    # ACT owns all of group 3 (its round-31 tile is ACT's last product, so
    # the PE finishes the final round right as ACT finishes -- an earlier
    # last-ACT-round would leave the PE draining a multi-round backlog at
    # ~0.86us per round after the producers are already done), plus two
    # mid-stream group-2 extras.
    return g == 3 or (g == 2 and r in (10, 21))


ACT_TILES = [tile_of(g, r) for g in range(NG) for r in range(NR) if is_act(g, r)]

# ---------------------------------------------------------------------------
# Workaround for the installed walrus accepting at most ONE sync-wait command
# per TPB instruction: split multi-wait instructions into preceding
# single-wait EventSemaphore instructions on the same engine.
# ---------------------------------------------------------------------------


def _fix_bir_multiwait(bir_json: bytes) -> bytes:
    d = orjson.loads(bir_json)
    counter = 0
    for fn in d.get("functions", []):
        stack = list(fn.get("blocks", []))
        while stack:
            block = stack.pop()
            stack.extend(block.get("blocks", []))
            new_insts = []
            for inst in block.get("instructions", []):
                sync = inst.get("sync_info") or {}
                waits = sync.get("on_wait") or []
                if len(waits) > 1:
                    for w in waits[:-1]:
                        counter += 1
                        new_insts.append({
                            "debug": inst.get("debug", 0),
                            "engine": inst.get("engine"),
                            "ins": [],
                            "name": f"esw_fix_{counter}",
                            "opcode": "EventSemaphore",
                            "outs": [],
                            "sync_info": {"on_update": [], "on_wait": [w]},
                        })
                    sync["on_wait"] = [waits[-1]]
                new_insts.append(inst)
            block["instructions"] = new_insts
    return orjson.dumps(d)


_patched = False


def _install_bir_fix():
    global _patched
    if _patched:
        return
    _patched = True
    import concourse.bass_utils as bu
    import concourse.bass2jax as b2j

    orig = bu.compile_bir_kernel

    def patched(bir_json, tmpdir, neff_name="file.neff"):
        if isinstance(bir_json, str):
            bir_json = bir_json.encode()
        return orig(_fix_bir_multiwait(bir_json), tmpdir, neff_name)

    bu.compile_bir_kernel = patched
    b2j.compile_bir_kernel = patched


# ---------------------------------------------------------------------------
# Kernel build
# ---------------------------------------------------------------------------

def build_kernel() -> bass.Bass:
    nc = bass.Bass()
    Sign = mybir.ActivationFunctionType.Sign

    # ONE wide critical load (6KB lines amortize the per-line DMA cost, and
    # a dma_start occupies its issuing engine for the whole transfer, so
    # fewer/wider is strictly better): [yb bf16 as f32-pairs | ycol | rcol |
    # scale_b | indh_b]
    ALLF_W = JSHARD // 2 + NT + NT + NT + NT
    allf = nc.dram_tensor("allf", [128, ALLF_W], F32, kind="ExternalInput")
    # rest: [r_pf | e_pf | e_f | w | sel] -- off the critical path, rides
    # the slow gpsimd software-dge queue in parallel
    REST_W = 16 + 16 + NT + 1024 + 2
    rest = nc.dram_tensor("rest", [128, REST_W], F32, kind="ExternalInput")
    out = nc.dram_tensor("out", [1, 3], F32, kind="ExternalOutput")

    with tile.TileContext(nc) as tc:
        with (
            tc.tile_pool(name="const", bufs=1) as const,
            tc.tile_pool(name="masks", bufs=26) as masks,
            tc.tile_pool(name="psacc", bufs=1, space="PSUM") as psacc,
            tc.tile_pool(name="psaux", bufs=1, space="PSUM") as psaux,
        ):
            # ---- DVE-local init first (no input deps; overlaps the DMAs)
            ones_col = const.tile([128, 1], F32)
            nc.vector.memset(ones_col, 1.0)
            zeros_32 = const.tile([128, 32], BF16)
            nc.vector.memset(zeros_32, 0.0)
            ones_b = const.tile([128, 512], BF16)
            nc.vector.memset(ones_b, 1.0)
            lhsT = const.tile([128, 3, NT], BF16)

            # ---- critical-path loads: the wide critical pack is split
            # across BOTH hw-dge queues so the two ~3us transfers run in
            # parallel (a dma_start occupies its issuing engine for the
            # whole transfer); rest rides the slow gpsimd software-dge
            # queue fully in parallel
            allf_sb = const.tile([128, ALLF_W], F32)
            nc.sync.dma_start(out=allf_sb[:, 0:896], in_=allf[:, 0:896])
            nc.scalar.dma_start(out=allf_sb[:, 896:ALLF_W],
                                in_=allf[:, 896:ALLF_W])
            rest_sb = const.tile([128, REST_W], F32)
            nc.gpsimd.dma_start(out=rest_sb, in_=rest[:, :])
            o = JSHARD // 2
            yb = allf_sb[:, 0:o].bitcast(BF16)         # j-side codes
            ycol_sb = allf_sb[:, o:o + NT]; o += NT    # code_i + delta (f32)
            rcol_sb = allf_sb[:, o:o + NT]; o += NT    # risk_pred col-major
            scale_b = allf_sb[:, o:o + NT]; o += NT    # 0.5 on ACT cols else 1
            indh_b = allf_sb[:, o:o + NT]; o += NT     # 0.5 on ACT cols else 0
            o = 0
            rpf_sb = rest_sb[:, o:o + 16]; o += 16
            epf_f = rest_sb[:, o:o + 16]; o += 16
            e_f = rest_sb[:, o:o + NT]; o += NT
            w_sb = rest_sb[:, o:o + 1024]; o += 1024
            sel_sb = rest_sb[:, o:o + 2]; o += 2       # group-fold selectors

            exp_sb = const.tile([128, NT], F32)
            nc.scalar.activation(exp_sb, rcol_sb, mybir.ActivationFunctionType.Exp)

            # ---- main loop: mask tiles + column-tiled matmul accumulation
            # acc group g lives at partitions [32g, 32g+32), banks by jj chunk
            acc = psacc.tile([128, NJJ * 512], F32)

            # start-of-chain init matmuls: zero weights, start=True.  They
            # zero ALL 32 partitions of each group's PSUM block and set
            # has_written, so the thin [3, 512] main matmuls can accumulate
            # with start=False and the epilogue can read PSUM full-width.
            # (Also serves as the PE HAM warm-up.)
            for g in range(NG):
                for jj in range(NJJ):
                    nc.tensor.matmul(
                        acc[32 * g:32 * g + 32, 512 * jj:512 * (jj + 1)],
                        zeros_32, ones_b,
                        start=True, stop=False,
                        tile_position=(0, 32 * g), skip_group_check=True,
                    )

            lhsT_va = const.tile([128, 32], BF16)
            vh = const.tile([128, NT], F32)
            vred = const.tile([128, 1], F32)
            vhi32 = const.tile([128, 1], F32)
            vlo = const.tile([128, 1], F32)
            hi32 = const.tile([128, NT], F32)
            lo32 = const.tile([128, NT], F32)
            vec3 = const.tile([128, 3], F32)

            for r in range(NR):
                mt = {}
                for g in range(NG):
                    t = tile_of(g, r)
                    m = masks.tile([128, JSHARD], BF16)
                    mt[g] = m
                    if is_act(g, r):
                        nc.scalar.activation(
                            m, yb, Sign, bias=ycol_sb[:, t:t + 1], scale=-1.0,
                        )
                    else:
                        nc.vector.tensor_scalar(
                            out=m, in0=yb,
                            scalar1=ycol_sb[:, t:t + 1], scalar2=None,
                            op0=mybir.AluOpType.is_le,
                        )
                if r == 0:
                    # lhsT rows 0..2 = scale * [exp_hi | exp_lo | ones], bf16
                    # (emitted after the first masks so they head the DVE
                    # queue, but before any matmul reads lhsT)
                    nc.vector.tensor_copy(lhsT[:, 0, :], exp_sb)   # bf16(exp)
                    nc.vector.tensor_copy(hi32, lhsT[:, 0, :])     # back to f32
                    nc.vector.tensor_sub(lo32, exp_sb, hi32)       # f32 residual
                    nc.vector.tensor_mul(lhsT[:, 0, :], hi32, scale_b)
                    nc.vector.tensor_mul(lhsT[:, 1, :], lo32, scale_b)
                    nc.vector.tensor_copy(lhsT[:, 2, :], scale_b)
                # last round goes jj-outer so the jj<2 chains stop first and
                # the epilogue's first-half copy can begin ~0.7us earlier
                if r == NR - 1:
                    mmorder = [(g, jj) for jj in range(NJJ) for g in range(NG)]
                else:
                    mmorder = [(g, jj) for g in range(NG) for jj in range(NJJ)]
                for g, jj in mmorder:
                    t = tile_of(g, r)
                    nc.tensor.matmul(
                        acc[32 * g:32 * g + 3, 512 * jj:512 * (jj + 1)],
                        lhsT[:, :, t],
                        mt[g][:, 512 * jj:512 * (jj + 1)],
                        start=False,
                        stop=(r == NR - 1),
                        tile_position=(0, 32 * g),
                        skip_group_check=True,
                    )
                # deferred off-critical DVE prologue work (keeps the first
                # mask tiles at the head of the DVE queue)
                if r == 2:
                    nc.vector.memset(lhsT_va, 0.0)
                    nc.vector.tensor_mul(vh, exp_sb, indh_b)
                    nc.vector.tensor_reduce(
                        out=vred, in_=vh, axis=mybir.AxisListType.X,
                        op=mybir.AluOpType.add)
                elif r == 3:
                    nc.vector.tensor_copy(lhsT_va[:, 0:1], vred)  # vred_hi
                    nc.vector.tensor_copy(vhi32, lhsT_va[:, 0:1])
                    nc.vector.tensor_sub(vlo, vred, vhi32)
                    nc.vector.tensor_copy(lhsT_va[:, 1:2], vlo)   # vred_lo
                    nc.vector.memset(lhsT_va[:, 2:3], DEN_ROW)
                elif r == 8:
                    nc.vector.tensor_reduce(
                        out=vec3[:, 0:1], in_=e_f, axis=mybir.AxisListType.X,
                        op=mybir.AluOpType.add)
                elif r == 6:
                    # correction matmuls, mid-stream: add [V_half_hi,
                    # V_half_lo, NACT/2] (summed over partitions by the PE)
                    # into the group-0 rows for every column
                    for jj in range(NJJ):
                        nc.tensor.matmul(
                            acc[0:32, 512 * jj:512 * (jj + 1)],
                            lhsT_va, ones_b,
                            start=False, stop=False,
                            tile_position=(0, 0), skip_group_check=True,
                        )

            # ---- epilogue: PSUM -> SBUF staging (chunked, both engines),
            # selector matmul folds 4 groups' (hi+lo) and den rows into
            # [num | den], 2 contiguous-dest scatter DMAs into pf layout.
            # pf mapping: x_pf[p, c] = x_shard[16*p + c]
            F32R = mybir.dt.float32r
            sel_r = const.tile([128, 2], F32R)
            nc.vector.tensor_copy(sel_r, sel_sb)
            nd_all = const.tile([128, NJJ * 512], F32R)
            fold = psaux.tile([2, NJJ * 512], F32, name="fold")
            lnnd = const.tile([2, NJJ * 512], F32)
            lnn = const.tile([128, 16], F32)
            lnd = const.tile([128, 16], F32)
            s1 = const.tile([128, 16], F32)
            s2 = const.tile([128, 16], F32)
            s3 = const.tile([128, 16], F32)
            # both staging copies run concurrently (vector h0, scalar h1 --
            # scalar's Ln work only starts once fold-h0 exists anyway)
            nc.vector.tensor_copy(nd_all[:, 0:1024], acc[:, 0:1024])
            nc.scalar.copy(nd_all[:, 1024:2048], acc[:, 1024:2048])
            for h in range(2):
                cs = slice(1024 * h, 1024 * (h + 1))
                # f32r: single-pass reduced-precision fp32 matmul (fp22
                # mantissa -- plenty for the 2e-2 budget, 2x faster)
                for jj in (2 * h, 2 * h + 1):
                    nc.tensor.matmul(
                        fold[:, 512 * jj:512 * (jj + 1)],
                        sel_r, nd_all[:, 512 * jj:512 * (jj + 1)],
                        start=True, stop=True, skip_group_check=True,
                    )
                # Ln straight off the PSUM fold rows (both rows per chunk),
                # then scatter this half into pf partitions [64h, 64h+64)
                # (both scatters on the otherwise-idle sync queue so the
                # scalar engine proceeds straight to the next Ln), and run
                # the [64, 16] final math for this half immediately
                nc.scalar.activation(lnnd[:, cs], fold[:, cs],
                                     mybir.ActivationFunctionType.Ln)
                ps = slice(64 * h, 64 * (h + 1))
                nc.sync.dma_start(out=lnn[ps, :], in_=lnnd[0:1, cs])
                nc.sync.dma_start(out=lnd[ps, :], in_=lnnd[1:2, cs])
                nc.vector.tensor_sub(s1[ps, :], rpf_sb[ps, :], lnn[ps, :])
                nc.vector.scalar_tensor_tensor(
                    out=s2[ps, :], in0=s1[ps, :], scalar=1.0, in1=lnd[ps, :],
                    op0=mybir.AluOpType.mult, op1=mybir.AluOpType.add)
                nc.vector.scalar_tensor_tensor(
                    out=s3[ps, :], in0=s2[ps, :], scalar=1.0,
                    in1=epf_f[ps, :],
                    op0=mybir.AluOpType.mult, op1=mybir.AluOpType.mult,
                    accum_out=vec3[ps, 2:3])

            # ---- W frobenius^2 (gates only the final sums matmul)
            w2d = const.tile([128, 1024], F32)
            nc.scalar.activation(
                w2d, w_sb, mybir.ActivationFunctionType.Square,
                accum_out=vec3[:, 1:2],
            )

            # ---- cross-partition fold: [e_sum, w_ssq, t_sum] into one row
            # (reuses a slice of the fold tile; WAR deps order it after nd2)
            sums = fold[0:1, 0:3]
            nc.tensor.matmul(sums, ones_col, vec3[:, :], start=True,
                             stop=True, skip_group_check=True)
            res3 = const.tile([1, 3], F32)
            nc.vector.tensor_copy(res3, sums)
            nc.sync.dma_start(out=out[:, :], in_=res3)

    return nc


_nc_cache = None


def _get_nc():
    global _nc_cache
    if _nc_cache is None:
        _install_bir_fix()
        _nc_cache = build_kernel()
    return _nc_cache


def make_in_maps(risk_pred, y, e, W):
    """Host-side sharding: slice/reshape/encode the full inputs per core."""
    yflat = y.reshape(-1)
    # monotone distinct bf16 codes: rank -> bf16 bit pattern (+0x2000 keeps
    # every code and its successor a normal number in [2^-63, 2^64], so
    # all pairwise differences are far from f32 under/overflow)
    order = np.argsort(yflat, kind="stable")
    ranks = np.empty(N, np.uint16)
    ranks[order] = np.arange(N, dtype=np.uint16)
    codes_u16 = (ranks + np.uint16(0x2000)).astype(np.uint16)
    codes_bf16 = codes_u16.view(ml_dtypes.bfloat16)
    codes_f32 = codes_bf16.astype(np.float32)
    nxt_f32 = (codes_u16 + np.uint16(1)).view(ml_dtypes.bfloat16).astype(np.float32)
    # row-side codes get +quarter-gap so the diagonal compare is strict (+1)
    ycol_delta = codes_f32 + 0.25 * (nxt_f32 - codes_f32)

    ycol = ycol_delta.reshape(NT, 128).T                     # [p, t]
    rcol = risk_pred.reshape(NT, 128).T.astype(np.float32)
    ef = e.astype(np.float32).reshape(NT, 128).T
    w_flat = W.reshape(128, 1024).astype(np.float32)
    act_mask = np.zeros(NT, np.float32)
    act_mask[ACT_TILES] = 1.0
    scale_b = np.tile(1.0 - 0.5 * act_mask, (128, 1)).astype(np.float32)
    indh_b = np.tile(0.5 * act_mask, (128, 1)).astype(np.float32)
    # group-fold selector: col 0 sums the hi+lo rows (p%32 in {0,1}) of the
    # 4 PSUM groups, col 1 sums the den rows (p%32 == 2)
    p = np.arange(128)
    sel = np.stack([((p % 32) <= 1), ((p % 32) == 2)], axis=1).astype(np.float32)

    crit = np.ascontiguousarray(np.concatenate(
        [ycol, rcol, scale_b, indh_b], axis=1), dtype=np.float32)

    in_maps = []
    for c in range(NCORES):
        j0 = c * JSHARD
        rsh = risk_pred.reshape(-1)[j0:j0 + JSHARD]
        esh = e.astype(np.float32).reshape(-1)[j0:j0 + JSHARD]
        r_pf = rsh.reshape(128, 16).astype(np.float32)
        e_pf = esh.reshape(128, 16)
        rest = np.ascontiguousarray(np.concatenate(
            [r_pf, e_pf, ef, w_flat, sel], axis=1), dtype=np.float32)
        yb = np.ascontiguousarray(
            np.broadcast_to(codes_bf16[j0:j0 + JSHARD], (128, JSHARD)))
        allf = np.ascontiguousarray(np.concatenate(
            [yb.view(np.float32), crit], axis=1), dtype=np.float32)
        in_maps.append(dict(allf=allf, rest=rest))
    return in_maps


def kernel(risk_pred, y, e, W, **run_kwargs):
    nc = _get_nc()
    in_maps = make_in_maps(
        np.asarray(risk_pred, np.float32),
        np.asarray(y, np.float32),
        np.asarray(e, np.int32),
        np.asarray(W, np.float32),
    )
    result = run_bass_kernel_spmd(nc, in_maps, core_ids=list(range(NCORES)),
                                  **run_kwargs)
    # gather/unshard: t_sum adds across cores; e_sum and w_ssq are computed
    # from replicated inputs (identical on every core)
    t_total = np.float32(0.0)
    for r in result.results:
        t_total = np.float32(t_total + r["out"][0, 2])
    e_sum = np.float32(result.results[0]["out"][0, 0])
    w_ssq = np.float32(result.results[0]["out"][0, 1])
    total = np.float32(-t_total / e_sum + np.float32(0.01) * np.sqrt(w_ssq))
    kernel.last_result = result
    return np.asarray(total, np.float32)


# revision 77
# speedup vs baseline: 1.1019x; 1.0078x over previous
"""Cox partial-likelihood NegativeLogLikelihood loss on 8 Trainium2 cores.

reference:
    mask[i, j] = (y[j] <= y[i])                       # (N, N)
    num[j] = sum_i exp(r_i) * mask[i, j]
    den[j] = sum_i mask[i, j]
    loss = -sum_j e_j * (r_j - log(num_j / den_j)) / sum_j e_j + 0.01 * ||W||_F

Strategy: shard columns j across the 8 cores (each core owns 2048 columns).
The N x 2048 mask is materialized on-chip in [128, 2048] tiles and contracted
on the TensorEngine against lhsT = [exp_hi, exp_lo, 1, 0...] (bf16 Dekker
split, padded to 32 rows) into PSUM.

Perf structure:
  * y is re-encoded on the host as monotone bf16 codes (rank -> bf16 bit
    pattern + 0x2000), so comparisons are exact in bf16 and the DVE
    tensor_scalar(is_le) compare runs in the 4x perf mode (~0.74us per
    [128, 2048] tile vs ~1.1us for the f32 compare).
  * Row-side code copies carry a +quarter-ulp offset so code_i' > code_j
    strictly for i == j: the ScalarE Sign producer yields exactly +/-1 and
    all tie/diagonal corrections vanish.  DVE produces 93 tiles, ACT 35
    (sign-encoded with halved weights; ~2.0us/tile), balancing the engines.
  * Matmuls are column-tiled 4 ways (tile_position=(0, 32g)): four thin-M
    matmuls execute concurrently in disjoint 32-column strips of the PE
    array.  PSUM group g accumulates i-tiles 32g..32g+31 at partitions
    32g..32g+31 (rows 3..31 zero-padded so the epilogue can read PSUM
    full-width).
  * The sign-encoding corrections (+V_half into hi/lo rows, +NACT/2 per
    partition into den) are folded in by one extra matmul per jj chunk
    against an all-ones rhs -- no scalar broadcast round-trip.
  * Epilogue: PSUM -> SBUF copies chunked across scalar+vector, a selector
    matmul folds the 4 groups' rows into [num | den], 2 contiguous-dest
    scatter DMAs redistribute to [128, 16] pf layout, and each core emits
    [e_sum, w_ssq, t_sum]; the host unshard sums t over cores and applies
    -t/e + 0.01*sqrt(w_ssq).
"""
import numpy as np
import ml_dtypes
import orjson

import concourse.bass as bass
import concourse.tile as tile
import concourse.mybir as mybir
from concourse.bass_utils import run_bass_kernel_spmd

F32 = mybir.dt.float32
BF16 = mybir.dt.bfloat16

N = 16384
NCORES = 8
JSHARD = N // NCORES            # 2048 columns per core
NT = N // 128                   # 128 i-tiles of 128 rows
NG = 4                          # PE column-strip groups
NR = NT // NG                   # 32 i-tiles (rounds) per group
NJJ = JSHARD // 512             # 4 matmul column chunks per core
NACT = 34                       # ACT-produced tiles (sign-encoded)
DEN_ROW = float(NACT) / 2.0     # per-partition den correction row


def tile_of(g, r):
    return 32 * g + r


def is_act(g, r):
    # ACT owns all of group 3 (its round-31 tile is ACT's last product, so
    # the PE finishes the final round right as ACT finishes -- an earlier
    # last-ACT-round would leave the PE draining a multi-round backlog at
    # ~0.86us per round after the producers are already done), plus two
    # mid-stream group-2 extras.
    return g == 3 or (g == 2 and r in (10, 21))


ACT_TILES = [tile_of(g, r) for g in range(NG) for r in range(NR) if is_act(g, r)]

# ---------------------------------------------------------------------------
# Workaround for the installed walrus accepting at most ONE sync-wait command
# per TPB instruction: split multi-wait instructions into preceding
# single-wait EventSemaphore instructions on the same engine.
# ---------------------------------------------------------------------------


def _fix_bir_multiwait(bir_json: bytes) -> bytes:
    d = orjson.loads(bir_json)
    counter = 0
    for fn in d.get("functions", []):
        stack = list(fn.get("blocks", []))
        while stack:
            block = stack.pop()
            stack.extend(block.get("blocks", []))
            new_insts = []
            for inst in block.get("instructions", []):
                sync = inst.get("sync_info") or {}
                waits = sync.get("on_wait") or []
                if len(waits) > 1:
                    for w in waits[:-1]:
                        counter += 1
                        new_insts.append({
                            "debug": inst.get("debug", 0),
                            "engine": inst.get("engine"),
                            "ins": [],
                            "name": f"esw_fix_{counter}",
                            "opcode": "EventSemaphore",
                            "outs": [],
                            "sync_info": {"on_update": [], "on_wait": [w]},
                        })
                    sync["on_wait"] = [waits[-1]]
                new_insts.append(inst)
            block["instructions"] = new_insts
    return orjson.dumps(d)


_patched = False


def _install_bir_fix():
    global _patched
    if _patched:
        return
    _patched = True
    import concourse.bass_utils as bu
    import concourse.bass2jax as b2j

    orig = bu.compile_bir_kernel

    def patched(bir_json, tmpdir, neff_name="file.neff"):
        if isinstance(bir_json, str):
            bir_json = bir_json.encode()
        return orig(_fix_bir_multiwait(bir_json), tmpdir, neff_name)

    bu.compile_bir_kernel = patched
    b2j.compile_bir_kernel = patched


# ---------------------------------------------------------------------------
# Kernel build
# ---------------------------------------------------------------------------

def build_kernel() -> bass.Bass:
    nc = bass.Bass()
    Sign = mybir.ActivationFunctionType.Sign

    # ONE wide critical load (6KB lines amortize the per-line DMA cost, and
    # a dma_start occupies its issuing engine for the whole transfer, so
    # fewer/wider is strictly better): [yb bf16 as f32-pairs | ycol | rcol |
    # scale_b | indh_b]
    ALLF_W = JSHARD // 2 + NT + NT + NT + NT
    allf = nc.dram_tensor("allf", [128, ALLF_W], F32, kind="ExternalInput")
    # rest: [r_pf | e_pf | e_f | w | sel] -- off the critical path, rides
    # the slow gpsimd software-dge queue in parallel
    REST_W = 16 + 16 + NT + 1024 + 2
    rest = nc.dram_tensor("rest", [128, REST_W], F32, kind="ExternalInput")
    out = nc.dram_tensor("out", [1, 3], F32, kind="ExternalOutput")

    with tile.TileContext(nc) as tc:
        with (
            tc.tile_pool(name="const", bufs=1) as const,
            tc.tile_pool(name="masks", bufs=26) as masks,
            tc.tile_pool(name="psacc", bufs=1, space="PSUM") as psacc,
            tc.tile_pool(name="psaux", bufs=1, space="PSUM") as psaux,
        ):
            # ---- DVE-local init first (no input deps; overlaps the DMAs)
            ones_col = const.tile([128, 1], F32)
            nc.vector.memset(ones_col, 1.0)
            zeros_32 = const.tile([128, 32], BF16)
            nc.vector.memset(zeros_32, 0.0)
            ones_b = const.tile([128, 512], BF16)
            nc.vector.memset(ones_b, 1.0)
            lhsT = const.tile([128, 3, NT], BF16)

            # ---- critical-path loads: the wide critical pack is split
            # across BOTH hw-dge queues so the two ~3us transfers run in
            # parallel (a dma_start occupies its issuing engine for the
            # whole transfer); rest rides the slow gpsimd software-dge
            # queue fully in parallel
            allf_sb = const.tile([128, ALLF_W], F32)
            nc.sync.dma_start(out=allf_sb[:, 0:896], in_=allf[:, 0:896])
            nc.scalar.dma_start(out=allf_sb[:, 896:ALLF_W],
                                in_=allf[:, 896:ALLF_W])
            rest_sb = const.tile([128, REST_W], F32)
            nc.gpsimd.dma_start(out=rest_sb, in_=rest[:, :])
            o = JSHARD // 2
            yb = allf_sb[:, 0:o].bitcast(BF16)         # j-side codes
            ycol_sb = allf_sb[:, o:o + NT]; o += NT    # code_i + delta (f32)
            rcol_sb = allf_sb[:, o:o + NT]; o += NT    # risk_pred col-major
            scale_b = allf_sb[:, o:o + NT]; o += NT    # 0.5 on ACT cols else 1
            indh_b = allf_sb[:, o:o + NT]; o += NT     # 0.5 on ACT cols else 0
            o = 0
            rpf_sb = rest_sb[:, o:o + 16]; o += 16
            epf_f = rest_sb[:, o:o + 16]; o += 16
            e_f = rest_sb[:, o:o + NT]; o += NT
            w_sb = rest_sb[:, o:o + 1024]; o += 1024
            sel_sb = rest_sb[:, o:o + 2]; o += 2       # group-fold selectors

            exp_sb = const.tile([128, NT], F32)
            nc.scalar.activation(exp_sb, rcol_sb, mybir.ActivationFunctionType.Exp)

            # ---- main loop: mask tiles + column-tiled matmul accumulation
            # acc group g lives at partitions [32g, 32g+32), banks by jj chunk
            acc = psacc.tile([128, NJJ * 512], F32)

            # start-of-chain init matmuls: zero weights, start=True.  They
            # zero ALL 32 partitions of each group's PSUM block and set
            # has_written, so the thin [3, 512] main matmuls can accumulate
            # with start=False and the epilogue can read PSUM full-width.
            # (Also serves as the PE HAM warm-up.)
            for g in range(NG):
                for jj in range(NJJ):
                    nc.tensor.matmul(
                        acc[32 * g:32 * g + 32, 512 * jj:512 * (jj + 1)],
                        zeros_32, ones_b,
                        start=True, stop=False,
                        tile_position=(0, 32 * g), skip_group_check=True,
                    )

            lhsT_va = const.tile([128, 32], BF16)
            vh = const.tile([128, NT], F32)
            vred = const.tile([128, 1], F32)
            vhi32 = const.tile([128, 1], F32)
            vlo = const.tile([128, 1], F32)
            hi32 = const.tile([128, NT], F32)
            lo32 = const.tile([128, NT], F32)
            vec3 = const.tile([128, 3], F32)

            for r in range(NR):
                mt = {}
                for g in range(NG):
                    t = tile_of(g, r)
                    m = masks.tile([128, JSHARD], BF16)
                    mt[g] = m
                    if is_act(g, r):
                        nc.scalar.activation(
                            m, yb, Sign, bias=ycol_sb[:, t:t + 1], scale=-1.0,
                        )
                    else:
                        nc.vector.tensor_scalar(
                            out=m, in0=yb,
                            scalar1=ycol_sb[:, t:t + 1], scalar2=None,
                            op0=mybir.AluOpType.is_le,
                        )
                if r == 0:
                    # lhsT rows 0..2 = scale * [exp_hi | exp_lo | ones], bf16
                    # (emitted after the first masks so they head the DVE
                    # queue, but before any matmul reads lhsT)
                    nc.vector.tensor_copy(lhsT[:, 0, :], exp_sb)   # bf16(exp)
                    nc.vector.tensor_copy(hi32, lhsT[:, 0, :])     # back to f32
                    nc.vector.tensor_sub(lo32, exp_sb, hi32)       # f32 residual
                    nc.vector.tensor_mul(lhsT[:, 0, :], hi32, scale_b)
                    nc.vector.tensor_mul(lhsT[:, 1, :], lo32, scale_b)
                    nc.vector.tensor_copy(lhsT[:, 2, :], scale_b)
                # last round goes jj-outer so the jj<2 chains stop first and
                # the epilogue's first-half copy can begin ~0.7us earlier
                if r == NR - 1:
                    mmorder = [(g, jj) for jj in range(NJJ) for g in range(NG)]
                else:
                    mmorder = [(g, jj) for g in range(NG) for jj in range(NJJ)]
                for g, jj in mmorder:
                    t = tile_of(g, r)
                    nc.tensor.matmul(
                        acc[32 * g:32 * g + 3, 512 * jj:512 * (jj + 1)],
                        lhsT[:, :, t],
                        mt[g][:, 512 * jj:512 * (jj + 1)],
                        start=False,
                        stop=(r == NR - 1),
                        tile_position=(0, 32 * g),
                        skip_group_check=True,
                    )
                # deferred off-critical DVE prologue work (keeps the first
                # mask tiles at the head of the DVE queue)
                if r == 2:
                    nc.vector.memset(lhsT_va, 0.0)
                    nc.vector.tensor_mul(vh, exp_sb, indh_b)
                    nc.vector.tensor_reduce(
                        out=vred, in_=vh, axis=mybir.AxisListType.X,
                        op=mybir.AluOpType.add)
                elif r == 3:
                    nc.vector.tensor_copy(lhsT_va[:, 0:1], vred)  # vred_hi
                    nc.vector.tensor_copy(vhi32, lhsT_va[:, 0:1])
                    nc.vector.tensor_sub(vlo, vred, vhi32)
                    nc.vector.tensor_copy(lhsT_va[:, 1:2], vlo)   # vred_lo
                    nc.vector.memset(lhsT_va[:, 2:3], DEN_ROW)
                elif r == 8:
                    nc.vector.tensor_reduce(
                        out=vec3[:, 0:1], in_=e_f, axis=mybir.AxisListType.X,
                        op=mybir.AluOpType.add)
                elif r == 6:
                    # correction matmuls, mid-stream: add [V_half_hi,
                    # V_half_lo, NACT/2] (summed over partitions by the PE)
                    # into the group-0 rows for every column
                    for jj in range(NJJ):
                        nc.tensor.matmul(
                            acc[0:32, 512 * jj:512 * (jj + 1)],
                            lhsT_va, ones_b,
                            start=False, stop=False,
                            tile_position=(0, 0), skip_group_check=True,
                        )

            # ---- epilogue: PSUM -> SBUF staging (chunked, both engines),
            # selector matmul folds 4 groups' (hi+lo) and den rows into
            # [num | den], 2 contiguous-dest scatter DMAs into pf layout.
            # pf mapping: x_pf[p, c] = x_shard[16*p + c]
            F32R = mybir.dt.float32r
            sel_r = const.tile([128, 2], F32R)
            nc.vector.tensor_copy(sel_r, sel_sb)
            nd_all = const.tile([128, NJJ * 512], F32R)
            fold = psaux.tile([2, NJJ * 512], F32, name="fold")
            lnnd = const.tile([2, NJJ * 512], F32)
            lnn = const.tile([128, 16], F32)
            lnd = const.tile([128, 16], F32)
            s1 = const.tile([128, 16], F32)
            s2 = const.tile([128, 16], F32)
            s3 = const.tile([128, 16], F32)
            # both staging copies run concurrently (vector h0, scalar h1 --
            # scalar's Ln work only starts once fold-h0 exists anyway)
            nc.vector.tensor_copy(nd_all[:, 0:1024], acc[:, 0:1024])
            nc.scalar.copy(nd_all[:, 1024:2048], acc[:, 1024:2048])
            for h in range(2):
                cs = slice(1024 * h, 1024 * (h + 1))
                # f32r: single-pass reduced-precision fp32 matmul (fp22
                # mantissa -- plenty for the 2e-2 budget, 2x faster)
                for jj in (2 * h, 2 * h + 1):
                    nc.tensor.matmul(
                        fold[:, 512 * jj:512 * (jj + 1)],
                        sel_r, nd_all[:, 512 * jj:512 * (jj + 1)],
                        start=True, stop=True, skip_group_check=True,
                    )
                # Ln straight off the PSUM fold rows (both rows per chunk),
                # then scatter this half into pf partitions [64h, 64h+64)
                # (both scatters on the otherwise-idle sync queue so the
                # scalar engine proceeds straight to the next Ln), and run
                # the [64, 16] final math for this half immediately
                nc.scalar.activation(lnnd[:, cs], fold[:, cs],
                                     mybir.ActivationFunctionType.Ln)
                ps = slice(64 * h, 64 * (h + 1))
                nc.sync.dma_start(out=lnn[ps, :], in_=lnnd[0:1, cs])
                nc.scalar.dma_start(out=lnd[ps, :], in_=lnnd[1:2, cs])
                nc.vector.tensor_sub(s1[ps, :], rpf_sb[ps, :], lnn[ps, :])
                nc.vector.scalar_tensor_tensor(
                    out=s2[ps, :], in0=s1[ps, :], scalar=1.0, in1=lnd[ps, :],
                    op0=mybir.AluOpType.mult, op1=mybir.AluOpType.add)
                nc.vector.scalar_tensor_tensor(
                    out=s3[ps, :], in0=s2[ps, :], scalar=1.0,
                    in1=epf_f[ps, :],
                    op0=mybir.AluOpType.mult, op1=mybir.AluOpType.mult,
                    accum_out=vec3[ps, 2:3])

            # ---- W frobenius^2 (gates only the final sums matmul)
            w2d = const.tile([128, 1024], F32)
            nc.scalar.activation(
                w2d, w_sb, mybir.ActivationFunctionType.Square,
                accum_out=vec3[:, 1:2],
            )

            # ---- cross-partition fold: [e_sum, w_ssq, t_sum] into one row
            # (reuses a slice of the fold tile; WAR deps order it after nd2)
            sums = fold[0:1, 0:3]
            nc.tensor.matmul(sums, ones_col, vec3[:, :], start=True,
                             stop=True, skip_group_check=True)
            res3 = const.tile([1, 3], F32)
            nc.vector.tensor_copy(res3, sums)
            nc.sync.dma_start(out=out[:, :], in_=res3)

    return nc


_nc_cache = None


def _get_nc():
    global _nc_cache
    if _nc_cache is None:
        _install_bir_fix()
        _nc_cache = build_kernel()
    return _nc_cache


def make_in_maps(risk_pred, y, e, W):
    """Host-side sharding: slice/reshape/encode the full inputs per core."""
    yflat = y.reshape(-1)
    # monotone distinct bf16 codes: rank -> bf16 bit pattern (+0x2000 keeps
    # every code and its successor a normal number in [2^-63, 2^64], so
    # all pairwise differences are far from f32 under/overflow)
    order = np.argsort(yflat, kind="stable")
    ranks = np.empty(N, np.uint16)
    ranks[order] = np.arange(N, dtype=np.uint16)
    codes_u16 = (ranks + np.uint16(0x2000)).astype(np.uint16)
    codes_bf16 = codes_u16.view(ml_dtypes.bfloat16)
    codes_f32 = codes_bf16.astype(np.float32)
    nxt_f32 = (codes_u16 + np.uint16(1)).view(ml_dtypes.bfloat16).astype(np.float32)
    # row-side codes get +quarter-gap so the diagonal compare is strict (+1)
    ycol_delta = codes_f32 + 0.25 * (nxt_f32 - codes_f32)

    ycol = ycol_delta.reshape(NT, 128).T                     # [p, t]
    rcol = risk_pred.reshape(NT, 128).T.astype(np.float32)
    ef = e.astype(np.float32).reshape(NT, 128).T
    w_flat = W.reshape(128, 1024).astype(np.float32)
    act_mask = np.zeros(NT, np.float32)
    act_mask[ACT_TILES] = 1.0
    scale_b = np.tile(1.0 - 0.5 * act_mask, (128, 1)).astype(np.float32)
    indh_b = np.tile(0.5 * act_mask, (128, 1)).astype(np.float32)
    # group-fold selector: col 0 sums the hi+lo rows (p%32 in {0,1}) of the
    # 4 PSUM groups, col 1 sums the den rows (p%32 == 2)
    p = np.arange(128)
    sel = np.stack([((p % 32) <= 1), ((p % 32) == 2)], axis=1).astype(np.float32)

    crit = np.ascontiguousarray(np.concatenate(
        [ycol, rcol, scale_b, indh_b], axis=1), dtype=np.float32)

    in_maps = []
    for c in range(NCORES):
        j0 = c * JSHARD
        rsh = risk_pred.reshape(-1)[j0:j0 + JSHARD]
        esh = e.astype(np.float32).reshape(-1)[j0:j0 + JSHARD]
        r_pf = rsh.reshape(128, 16).astype(np.float32)
        e_pf = esh.reshape(128, 16)
        rest = np.ascontiguousarray(np.concatenate(
            [r_pf, e_pf, ef, w_flat, sel], axis=1), dtype=np.float32)
        yb = np.ascontiguousarray(
            np.broadcast_to(codes_bf16[j0:j0 + JSHARD], (128, JSHARD)))
        allf = np.ascontiguousarray(np.concatenate(
            [yb.view(np.float32), crit], axis=1), dtype=np.float32)
        in_maps.append(dict(allf=allf, rest=rest))
    return in_maps


def kernel(risk_pred, y, e, W, **run_kwargs):
    nc = _get_nc()
    in_maps = make_in_maps(
        np.asarray(risk_pred, np.float32),
        np.asarray(y, np.float32),
        np.asarray(e, np.int32),
        np.asarray(W, np.float32),
    )
    result = run_bass_kernel_spmd(nc, in_maps, core_ids=list(range(NCORES)),
                                  **run_kwargs)
    # gather/unshard: t_sum adds across cores; e_sum and w_ssq are computed
    # from replicated inputs (identical on every core)
    t_total = np.float32(0.0)
    for r in result.results:
        t_total = np.float32(t_total + r["out"][0, 2])
    e_sum = np.float32(result.results[0]["out"][0, 0])
    w_ssq = np.float32(result.results[0]["out"][0, 1])
    total = np.float32(-t_total / e_sum + np.float32(0.01) * np.sqrt(w_ssq))
    kernel.last_result = result
    return np.asarray(total, np.float32)


# revision 81
# speedup vs baseline: 1.1030x; 1.0011x over previous
"""Cox partial-likelihood NegativeLogLikelihood loss on 8 Trainium2 cores.

reference:
    mask[i, j] = (y[j] <= y[i])                       # (N, N)
    num[j] = sum_i exp(r_i) * mask[i, j]
    den[j] = sum_i mask[i, j]
    loss = -sum_j e_j * (r_j - log(num_j / den_j)) / sum_j e_j + 0.01 * ||W||_F

Strategy: shard columns j across the 8 cores (each core owns 2048 columns).
The N x 2048 mask is materialized on-chip in [128, 2048] tiles and contracted
on the TensorEngine against lhsT = [exp_hi, exp_lo, 1, 0...] (bf16 Dekker
split, padded to 32 rows) into PSUM.

Perf structure:
  * y is re-encoded on the host as monotone bf16 codes (rank -> bf16 bit
    pattern + 0x2000), so comparisons are exact in bf16 and the DVE
    tensor_scalar(is_le) compare runs in the 4x perf mode (~0.74us per
    [128, 2048] tile vs ~1.1us for the f32 compare).
  * Row-side code copies carry a +quarter-ulp offset so code_i' > code_j
    strictly for i == j: the ScalarE Sign producer yields exactly +/-1 and
    all tie/diagonal corrections vanish.  DVE produces 94 tiles, ACT 34
    (sign-encoded with halved weights; ~1.9us/tile), balancing the engines.
  * Matmuls are column-tiled 4 ways (tile_position=(0, 32g)): four thin-M
    matmuls execute concurrently in disjoint 32-column strips of the PE
    array.  PSUM group g accumulates i-tiles 32g..32g+31 at partitions
    32g..32g+31 (rows 3..31 zero-padded so the epilogue can read PSUM
    full-width).
  * The sign-encoding corrections (+V_half into hi/lo rows, +NACT/2 per
    partition into den) are folded in by one extra matmul per jj chunk
    against an all-ones rhs -- no scalar broadcast round-trip.
  * Epilogue: PSUM -> SBUF copies chunked across scalar+vector, a selector
    matmul folds the 4 groups' rows into [num | den], 2 contiguous-dest
    scatter DMAs redistribute to [128, 16] pf layout, and each core emits
    [e_sum, w_ssq, t_sum]; the host unshard sums t over cores and applies
    -t/e + 0.01*sqrt(w_ssq).
"""
import numpy as np
import ml_dtypes
import orjson

import concourse.bass as bass
import concourse.tile as tile
import concourse.mybir as mybir
from concourse.bass_utils import run_bass_kernel_spmd

F32 = mybir.dt.float32
BF16 = mybir.dt.bfloat16

N = 16384
NCORES = 8
JSHARD = N // NCORES            # 2048 columns per core
NT = N // 128                   # 128 i-tiles of 128 rows
NG = 4                          # PE column-strip groups
NR = NT // NG                   # 32 i-tiles (rounds) per group
NJJ = JSHARD // 512             # 4 matmul column chunks per core
NACT = 33                       # ACT-produced tiles (sign-encoded)
DEN_ROW = float(NACT) / 2.0     # per-partition den correction row


def tile_of(g, r):
    return 32 * g + r


def is_act(g, r):
    # ACT owns all of group 3 (its round-31 tile is ACT's last product, so
    # the PE finishes the final round right as ACT finishes -- an earlier
    # last-ACT-round would leave the PE draining a multi-round backlog at
    # ~0.86us per round after the producers are already done), plus two
    # mid-stream group-2 extras.
    return g == 3 or (g == 2 and r == 16)


ACT_TILES = [tile_of(g, r) for g in range(NG) for r in range(NR) if is_act(g, r)]

# ---------------------------------------------------------------------------
# Workaround for the installed walrus accepting at most ONE sync-wait command
# per TPB instruction: split multi-wait instructions into preceding
# single-wait EventSemaphore instructions on the same engine.
# ---------------------------------------------------------------------------


def _fix_bir_multiwait(bir_json: bytes) -> bytes:
    d = orjson.loads(bir_json)
    counter = 0
    for fn in d.get("functions", []):
        stack = list(fn.get("blocks", []))
        while stack:
            block = stack.pop()
            stack.extend(block.get("blocks", []))
            new_insts = []
            for inst in block.get("instructions", []):
                sync = inst.get("sync_info") or {}
                waits = sync.get("on_wait") or []
                if len(waits) > 1:
                    for w in waits[:-1]:
                        counter += 1
                        new_insts.append({
                            "debug": inst.get("debug", 0),
                            "engine": inst.get("engine"),
                            "ins": [],
                            "name": f"esw_fix_{counter}",
                            "opcode": "EventSemaphore",
                            "outs": [],
                            "sync_info": {"on_update": [], "on_wait": [w]},
                        })
                    sync["on_wait"] = [waits[-1]]
                new_insts.append(inst)
            block["instructions"] = new_insts
    return orjson.dumps(d)


_patched = False


def _install_bir_fix():
    global _patched
    if _patched:
        return
    _patched = True
    import concourse.bass_utils as bu
    import concourse.bass2jax as b2j

    orig = bu.compile_bir_kernel

    def patched(bir_json, tmpdir, neff_name="file.neff"):
        if isinstance(bir_json, str):
            bir_json = bir_json.encode()
        return orig(_fix_bir_multiwait(bir_json), tmpdir, neff_name)

    bu.compile_bir_kernel = patched
    b2j.compile_bir_kernel = patched


# ---------------------------------------------------------------------------
# Kernel build
# ---------------------------------------------------------------------------

def build_kernel() -> bass.Bass:
    nc = bass.Bass()
    Sign = mybir.ActivationFunctionType.Sign

    # ONE wide critical load (6KB lines amortize the per-line DMA cost, and
    # a dma_start occupies its issuing engine for the whole transfer, so
    # fewer/wider is strictly better): [yb bf16 as f32-pairs | ycol | rcol |
    # scale_b | indh_b]
    ALLF_W = JSHARD // 2 + NT + NT + NT + NT
    allf = nc.dram_tensor("allf", [128, ALLF_W], F32, kind="ExternalInput")
    # rest: [r_pf | e_pf | e_f | w | sel] -- off the critical path, rides
    # the slow gpsimd software-dge queue in parallel
    REST_W = 16 + 16 + NT + 1024 + 2
    rest = nc.dram_tensor("rest", [128, REST_W], F32, kind="ExternalInput")
    out = nc.dram_tensor("out", [1, 3], F32, kind="ExternalOutput")

    with tile.TileContext(nc) as tc:
        with (
            tc.tile_pool(name="const", bufs=1) as const,
            tc.tile_pool(name="masks", bufs=26) as masks,
            tc.tile_pool(name="psacc", bufs=1, space="PSUM") as psacc,
            tc.tile_pool(name="psaux", bufs=1, space="PSUM") as psaux,
        ):
            # ---- DVE-local init first (no input deps; overlaps the DMAs)
            ones_col = const.tile([128, 1], F32)
            nc.vector.memset(ones_col, 1.0)
            zeros_32 = const.tile([128, 32], BF16)
            nc.vector.memset(zeros_32, 0.0)
            ones_b = const.tile([128, 512], BF16)
            nc.vector.memset(ones_b, 1.0)
            lhsT = const.tile([128, 3, NT], BF16)

            # ---- critical-path loads: the wide critical pack is split
            # across BOTH hw-dge queues so the two ~3us transfers run in
            # parallel (a dma_start occupies its issuing engine for the
            # whole transfer); rest rides the slow gpsimd software-dge
            # queue fully in parallel
            allf_sb = const.tile([128, ALLF_W], F32)
            nc.sync.dma_start(out=allf_sb[:, 0:896], in_=allf[:, 0:896])
            nc.scalar.dma_start(out=allf_sb[:, 896:ALLF_W],
                                in_=allf[:, 896:ALLF_W])
            rest_sb = const.tile([128, REST_W], F32)
            nc.gpsimd.dma_start(out=rest_sb, in_=rest[:, :])
            o = JSHARD // 2
            yb = allf_sb[:, 0:o].bitcast(BF16)         # j-side codes
            ycol_sb = allf_sb[:, o:o + NT]; o += NT    # code_i + delta (f32)
            rcol_sb = allf_sb[:, o:o + NT]; o += NT    # risk_pred col-major
            scale_b = allf_sb[:, o:o + NT]; o += NT    # 0.5 on ACT cols else 1
            indh_b = allf_sb[:, o:o + NT]; o += NT     # 0.5 on ACT cols else 0
            o = 0
            rpf_sb = rest_sb[:, o:o + 16]; o += 16
            epf_f = rest_sb[:, o:o + 16]; o += 16
            e_f = rest_sb[:, o:o + NT]; o += NT
            w_sb = rest_sb[:, o:o + 1024]; o += 1024
            sel_sb = rest_sb[:, o:o + 2]; o += 2       # group-fold selectors

            exp_sb = const.tile([128, NT], F32)
            nc.scalar.activation(exp_sb, rcol_sb, mybir.ActivationFunctionType.Exp)

            # ---- main loop: mask tiles + column-tiled matmul accumulation
            # acc group g lives at partitions [32g, 32g+32), banks by jj chunk
            acc = psacc.tile([128, NJJ * 512], F32)

            # start-of-chain init matmuls: zero weights, start=True.  They
            # zero ALL 32 partitions of each group's PSUM block and set
            # has_written, so the thin [3, 512] main matmuls can accumulate
            # with start=False and the epilogue can read PSUM full-width.
            # (Also serves as the PE HAM warm-up.)
            for g in range(NG):
                for jj in range(NJJ):
                    nc.tensor.matmul(
                        acc[32 * g:32 * g + 32, 512 * jj:512 * (jj + 1)],
                        zeros_32, ones_b,
                        start=True, stop=False,
                        tile_position=(0, 32 * g), skip_group_check=True,
                    )

            lhsT_va = const.tile([128, 32], BF16)
            vh = const.tile([128, NT], F32)
            vred = const.tile([128, 1], F32)
            vhi32 = const.tile([128, 1], F32)
            vlo = const.tile([128, 1], F32)
            hi32 = const.tile([128, NT], F32)
            lo32 = const.tile([128, NT], F32)
            vec3 = const.tile([128, 3], F32)

            for r in range(NR):
                mt = {}
                for g in range(NG):
                    t = tile_of(g, r)
                    m = masks.tile([128, JSHARD], BF16)
                    mt[g] = m
                    if is_act(g, r):
                        nc.scalar.activation(
                            m, yb, Sign, bias=ycol_sb[:, t:t + 1], scale=-1.0,
                        )
                    else:
                        nc.vector.tensor_scalar(
                            out=m, in0=yb,
                            scalar1=ycol_sb[:, t:t + 1], scalar2=None,
                            op0=mybir.AluOpType.is_le,
                        )
                if r == 0:
                    # lhsT rows 0..2 = scale * [exp_hi | exp_lo | ones], bf16
                    # (emitted after the first masks so they head the DVE
                    # queue, but before any matmul reads lhsT)
                    nc.vector.tensor_copy(lhsT[:, 0, :], exp_sb)   # bf16(exp)
                    nc.vector.tensor_copy(hi32, lhsT[:, 0, :])     # back to f32
                    nc.vector.tensor_sub(lo32, exp_sb, hi32)       # f32 residual
                    nc.vector.tensor_mul(lhsT[:, 0, :], hi32, scale_b)
                    nc.vector.tensor_mul(lhsT[:, 1, :], lo32, scale_b)
                    nc.vector.tensor_copy(lhsT[:, 2, :], scale_b)
                # last round goes jj-outer so the jj<2 chains stop first and
                # the epilogue's first-half copy can begin ~0.7us earlier
                if r == NR - 1:
                    mmorder = [(g, jj) for jj in range(NJJ) for g in range(NG)]
                else:
                    mmorder = [(g, jj) for g in range(NG) for jj in range(NJJ)]
                for g, jj in mmorder:
                    t = tile_of(g, r)
                    nc.tensor.matmul(
                        acc[32 * g:32 * g + 3, 512 * jj:512 * (jj + 1)],
                        lhsT[:, :, t],
                        mt[g][:, 512 * jj:512 * (jj + 1)],
                        start=False,
                        stop=(r == NR - 1),
                        tile_position=(0, 32 * g),
                        skip_group_check=True,
                    )
                # deferred off-critical DVE prologue work (keeps the first
                # mask tiles at the head of the DVE queue)
                if r == 2:
                    nc.vector.memset(lhsT_va, 0.0)
                    nc.vector.tensor_mul(vh, exp_sb, indh_b)
                    nc.vector.tensor_reduce(
                        out=vred, in_=vh, axis=mybir.AxisListType.X,
                        op=mybir.AluOpType.add)
                elif r == 3:
                    nc.vector.tensor_copy(lhsT_va[:, 0:1], vred)  # vred_hi
                    nc.vector.tensor_copy(vhi32, lhsT_va[:, 0:1])
                    nc.vector.tensor_sub(vlo, vred, vhi32)
                    nc.vector.tensor_copy(lhsT_va[:, 1:2], vlo)   # vred_lo
                    nc.vector.memset(lhsT_va[:, 2:3], DEN_ROW)
                elif r == 8:
                    nc.vector.tensor_reduce(
                        out=vec3[:, 0:1], in_=e_f, axis=mybir.AxisListType.X,
                        op=mybir.AluOpType.add)
                elif r == 6:
                    # correction matmuls, mid-stream: add [V_half_hi,
                    # V_half_lo, NACT/2] (summed over partitions by the PE)
                    # into the group-0 rows for every column
                    for jj in range(NJJ):
                        nc.tensor.matmul(
                            acc[0:32, 512 * jj:512 * (jj + 1)],
                            lhsT_va, ones_b,
                            start=False, stop=False,
                            tile_position=(0, 0), skip_group_check=True,
                        )

            # ---- epilogue: PSUM -> SBUF staging (chunked, both engines),
            # selector matmul folds 4 groups' (hi+lo) and den rows into
            # [num | den], 2 contiguous-dest scatter DMAs into pf layout.
            # pf mapping: x_pf[p, c] = x_shard[16*p + c]
            F32R = mybir.dt.float32r
            sel_r = const.tile([128, 2], F32R)
            nc.vector.tensor_copy(sel_r, sel_sb)
            nd_all = const.tile([128, NJJ * 512], F32R)
            fold = psaux.tile([2, NJJ * 512], F32, name="fold")
            lnnd = const.tile([2, NJJ * 512], F32)
            lnn = const.tile([128, 16], F32)
            lnd = const.tile([128, 16], F32)
            s1 = const.tile([128, 16], F32)
            s2 = const.tile([128, 16], F32)
            s3 = const.tile([128, 16], F32)
            # both staging copies run concurrently; scalar takes h0 so its
            # Ln-h0 follows engine-locally (and the jj-outer last round lets
            # h0's chains stop first)
            nc.scalar.copy(nd_all[:, 0:1024], acc[:, 0:1024])
            nc.vector.tensor_copy(nd_all[:, 1024:2048], acc[:, 1024:2048])
            for h in range(2):
                cs = slice(1024 * h, 1024 * (h + 1))
                # f32r: single-pass reduced-precision fp32 matmul (fp22
                # mantissa -- plenty for the 2e-2 budget, 2x faster)
                for jj in (2 * h, 2 * h + 1):
                    nc.tensor.matmul(
                        fold[:, 512 * jj:512 * (jj + 1)],
                        sel_r, nd_all[:, 512 * jj:512 * (jj + 1)],
                        start=True, stop=True, skip_group_check=True,
                    )
                # Ln straight off the PSUM fold rows (both rows per chunk),
                # then scatter this half into pf partitions [64h, 64h+64)
                # (both scatters on the otherwise-idle sync queue so the
                # scalar engine proceeds straight to the next Ln), and run
                # the [64, 16] final math for this half immediately
                nc.scalar.activation(lnnd[:, cs], fold[:, cs],
                                     mybir.ActivationFunctionType.Ln)
                ps = slice(64 * h, 64 * (h + 1))
                nc.sync.dma_start(out=lnn[ps, :], in_=lnnd[0:1, cs])
                nc.scalar.dma_start(out=lnd[ps, :], in_=lnnd[1:2, cs])
                nc.vector.tensor_sub(s1[ps, :], rpf_sb[ps, :], lnn[ps, :])
                nc.vector.scalar_tensor_tensor(
                    out=s2[ps, :], in0=s1[ps, :], scalar=1.0, in1=lnd[ps, :],
                    op0=mybir.AluOpType.mult, op1=mybir.AluOpType.add)
                nc.vector.scalar_tensor_tensor(
                    out=s3[ps, :], in0=s2[ps, :], scalar=1.0,
                    in1=epf_f[ps, :],
                    op0=mybir.AluOpType.mult, op1=mybir.AluOpType.mult,
                    accum_out=vec3[ps, 2:3])

            # ---- W frobenius^2 (gates only the final sums matmul)
            w2d = const.tile([128, 1024], F32)
            nc.scalar.activation(
                w2d, w_sb, mybir.ActivationFunctionType.Square,
                accum_out=vec3[:, 1:2],
            )

            # ---- cross-partition fold: [e_sum, w_ssq, t_sum] into one row
            # (reuses a slice of the fold tile; WAR deps order it after nd2)
            sums = fold[0:1, 0:3]
            nc.tensor.matmul(sums, ones_col, vec3[:, :], start=True,
                             stop=True, skip_group_check=True)
            res3 = const.tile([1, 3], F32)
            nc.vector.tensor_copy(res3, sums)
            nc.sync.dma_start(out=out[:, :], in_=res3)

    return nc


_nc_cache = None


def _get_nc():
    global _nc_cache
    if _nc_cache is None:
        _install_bir_fix()
        _nc_cache = build_kernel()
    return _nc_cache


def make_in_maps(risk_pred, y, e, W):
    """Host-side sharding: slice/reshape/encode the full inputs per core."""
    yflat = y.reshape(-1)
    # monotone distinct bf16 codes: rank -> bf16 bit pattern (+0x2000 keeps
    # every code and its successor a normal number in [2^-63, 2^64], so
    # all pairwise differences are far from f32 under/overflow)
    order = np.argsort(yflat, kind="stable")
    ranks = np.empty(N, np.uint16)
    ranks[order] = np.arange(N, dtype=np.uint16)
    codes_u16 = (ranks + np.uint16(0x2000)).astype(np.uint16)
    codes_bf16 = codes_u16.view(ml_dtypes.bfloat16)
    codes_f32 = codes_bf16.astype(np.float32)
    nxt_f32 = (codes_u16 + np.uint16(1)).view(ml_dtypes.bfloat16).astype(np.float32)
    # row-side codes get +quarter-gap so the diagonal compare is strict (+1)
    ycol_delta = codes_f32 + 0.25 * (nxt_f32 - codes_f32)

    ycol = ycol_delta.reshape(NT, 128).T                     # [p, t]
    rcol = risk_pred.reshape(NT, 128).T.astype(np.float32)
    ef = e.astype(np.float32).reshape(NT, 128).T
    w_flat = W.reshape(128, 1024).astype(np.float32)
    act_mask = np.zeros(NT, np.float32)
    act_mask[ACT_TILES] = 1.0
    scale_b = np.tile(1.0 - 0.5 * act_mask, (128, 1)).astype(np.float32)
    indh_b = np.tile(0.5 * act_mask, (128, 1)).astype(np.float32)
    # group-fold selector: col 0 sums the hi+lo rows (p%32 in {0,1}) of the
    # 4 PSUM groups, col 1 sums the den rows (p%32 == 2)
    p = np.arange(128)
    sel = np.stack([((p % 32) <= 1), ((p % 32) == 2)], axis=1).astype(np.float32)

    crit = np.ascontiguousarray(np.concatenate(
        [ycol, rcol, scale_b, indh_b], axis=1), dtype=np.float32)

    in_maps = []
    for c in range(NCORES):
        j0 = c * JSHARD
        rsh = risk_pred.reshape(-1)[j0:j0 + JSHARD]
        esh = e.astype(np.float32).reshape(-1)[j0:j0 + JSHARD]
        r_pf = rsh.reshape(128, 16).astype(np.float32)
        e_pf = esh.reshape(128, 16)
        rest = np.ascontiguousarray(np.concatenate(
            [r_pf, e_pf, ef, w_flat, sel], axis=1), dtype=np.float32)
        yb = np.ascontiguousarray(
            np.broadcast_to(codes_bf16[j0:j0 + JSHARD], (128, JSHARD)))
        allf = np.ascontiguousarray(np.concatenate(
            [yb.view(np.float32), crit], axis=1), dtype=np.float32)
        in_maps.append(dict(allf=allf, rest=rest))
    return in_maps


def kernel(risk_pred, y, e, W, **run_kwargs):
    nc = _get_nc()
    in_maps = make_in_maps(
        np.asarray(risk_pred, np.float32),
        np.asarray(y, np.float32),
        np.asarray(e, np.int32),
        np.asarray(W, np.float32),
    )
    result = run_bass_kernel_spmd(nc, in_maps, core_ids=list(range(NCORES)),
                                  **run_kwargs)
    # gather/unshard: t_sum adds across cores; e_sum and w_ssq are computed
    # from replicated inputs (identical on every core)
    t_total = np.float32(0.0)
    for r in result.results:
        t_total = np.float32(t_total + r["out"][0, 2])
    e_sum = np.float32(result.results[0]["out"][0, 0])
    w_ssq = np.float32(result.results[0]["out"][0, 1])
    total = np.float32(-t_total / e_sum + np.float32(0.01) * np.sqrt(w_ssq))
    kernel.last_result = result
    return np.asarray(total, np.float32)
